# revision 3
# baseline (speedup 1.0000x reference)
"""Trainium2 8-core attention kernel v7 (N=8192, D=512, Q==K shared projection).

fp8 DoubleRow formulation on top of the projection-free algebra:

    scores = SCALE * (Etilde E^T) + alpha,   Etilde = E G,  G = W_qk^T W_qk,
    alpha  = SCALE * E (W_qk^T b_qk)         (host-precomputed, f32 exact)
    attn @ V = (P @ E) W_v^T + b_v,          P = exp(scores), row-normalized

Per core: 1024 output rows; the 64 column chunks are host-ROTATED so the
core's own slab is always chunks 0..7.  Q==K makes the score diagonal
dominate the softmax (self-scores 7..14 vs off-diag |s|<4.6), so the own
slab runs in bf16 while the other 56 chunks use fp8: e4m3 operands for the
score/PV matmuls with perf_mode=DoubleRow (2 k-subtiles per instruction),
exp output in e5m2 (max P ~66 there, well under the 57344 ceiling; the
bf16 diag path holds the exp(14.4)=1.8e6 peaks).

Row sums ride the PV matmul as a ones-column (one [1,512] PSUM
accumulator), transposed back to [128,4] via 4 tiny PE transposes —
no per-chunk DVE work at all in steady state.
"""

import ml_dtypes
import numpy as np

import concourse.bass as bass
import concourse.mybir as mybir
import concourse.tile as tile
from concourse import bacc
from concourse.bass_utils import run_bass_kernel_spmd

N = 8192          # sequence length
F = 512           # features == head dim
D = 512
CORES = 8
NL = N // CORES   # local rows per core (1024)
SCALE = 1.0 / float(np.sqrt(D))

FC = F // 128     # 4 f-chunks
CC = N // 128     # 64 column chunks
OWN = NL // 128   # 8 own (bf16) chunks
NF8 = N - NL      # 7168 fp8 columns
FP8C = NF8 // 128  # 56 fp8 chunks
RB = NL // 512    # 2 row-blocks of 512

f32 = mybir.dt.float32
bf16 = mybir.dt.bfloat16
e4m3 = mybir.dt.float8e4
e5m2 = mybir.dt.float8e5
DR = mybir.MatmulPerfMode.DoubleRow

_NC = None
LAST_RESULT = None


def build_kernel():
    nc = bacc.Bacc(target_bir_lowering=False)

    et8d = nc.declare_dram_parameter("et8", [F, NF8], e4m3, isOutput=False)
    en8d = nc.declare_dram_parameter("en8", [128, FP8C * F], e4m3, isOutput=False)
    elg8d = nc.declare_dram_parameter("elg8", [F, NL], e4m3, isOutput=False)
    etld = nc.declare_dram_parameter("etl", [F, NL], bf16, isOutput=False)
    enld = nc.declare_dram_parameter("enl", [128, OWN * F], bf16, isOutput=False)
    elg16d = nc.declare_dram_parameter("elg16", [F, NL], bf16, isOutput=False)
    alphad = nc.declare_dram_parameter("alpha_t", [128, CC], f32, isOutput=False)
    wvTd = nc.declare_dram_parameter("wvT", [F, D], bf16, isOutput=False)
    bvd = nc.declare_dram_parameter("bv", [D], f32, isOutput=False)
    out = nc.declare_dram_parameter("out", [NL, D], f32, isOutput=True)

    with tile.TileContext(nc) as tc:
        with (
            tc.tile_pool(name="persist", bufs=1) as persist,
            tc.tile_pool(name="work", bufs=2) as work,
            tc.tile_pool(name="ps", bufs=2, space="PSUM") as ps,
        ):
            # ---- startup-critical DMAs first: fp8 score operands ----
            elg8 = persist.tile([128, FC, NL], e4m3)
            for fc in range(FC):
                nc.sync.dma_start(
                    out=elg8[:, fc:fc + 1, :],
                    in_=elg8d[fc * 128:(fc + 1) * 128, :].rearrange(
                        "p (o n) -> p o n", o=1))
            alpha_t = persist.tile([128, CC], f32)
            nc.sync.dma_start(out=alpha_t, in_=alphad[:, :])

            # et8 / en8 streamed in slabs of 7 chunks (896 cols)
            et8 = persist.tile([128, FC, NF8], e4m3)
            en8 = persist.tile([128, FP8C, F], e4m3)
            NSLAB = 8
            SW = NF8 // NSLAB          # 896 columns per slab
            SCH = FP8C // NSLAB        # 7 chunks per slab
            for sl in range(NSLAB):
                n0 = sl * SW
                for fc in range(FC):
                    nc.sync.dma_start(
                        out=et8[:, fc:fc + 1, n0:n0 + SW],
                        in_=et8d[fc * 128:(fc + 1) * 128, n0:n0 + SW].rearrange(
                            "p (o n) -> p o n", o=1))
                nc.gpsimd.dma_start(
                    out=en8[:, sl * SCH:(sl + 1) * SCH, :],
                    in_=en8d[:, n0 * 4:(n0 + SW) * 4].rearrange(
                        "p (c f) -> p c f", f=F))

            # ---- bf16 own-slab operands (consumed at the tail of each rb) ----
            etl = persist.tile([128, FC, NL], bf16)
            elg16 = persist.tile([128, FC, NL], bf16)
            enl = persist.tile([128, OWN, F], bf16)
            for fc in range(FC):
                nc.sync.dma_start(
                    out=etl[:, fc:fc + 1, :],
                    in_=etld[fc * 128:(fc + 1) * 128, :].rearrange(
                        "p (o n) -> p o n", o=1))
                nc.sync.dma_start(
                    out=elg16[:, fc:fc + 1, :],
                    in_=elg16d[fc * 128:(fc + 1) * 128, :].rearrange(
                        "p (o n) -> p o n", o=1))
            nc.gpsimd.dma_start(
                out=enl, in_=enld.rearrange("p (c f) -> p c f", f=F))

            wv = persist.tile([128, FC, D], bf16)
            for fc in range(FC):
                nc.sync.dma_start(
                    out=wv[:, fc:fc + 1, :],
                    in_=wvTd[fc * 128:(fc + 1) * 128, :].rearrange(
                        "p (o n) -> p o n", o=1))

            bv_bc = persist.tile([128, D], f32)
            bv_ap = bvd[:]
            nc.gpsimd.dma_start(out=bv_bc, in_=bass.AP(
                tensor=bv_ap.tensor, offset=bv_ap.offset,
                ap=[[0, 128], *bv_ap.ap]))

            ones16 = persist.tile([128, 1], bf16)
            nc.vector.memset(ones16, 1.0)
            ones8 = persist.tile([128, 2, 16], e4m3)
            nc.vector.memset(ones8, 1.0)
            id1 = persist.tile([1, 1], f32)
            nc.vector.memset(id1, 1.0)

            # ---- attention: 2 row-blocks of 512 local rows ----
            for rb in range(RB):
                r0 = rb * 512
                pvt_ps = [
                    ps.tile([128, 512], f32, tag="pvt_ps", bufs=4,
                            name=f"pvt{rb}_{fb}")
                    for fb in range(FC)
                ]
                l2_ps = ps.tile([1, 512], f32, tag="l2", bufs=1,
                                name=f"l2_{rb}")

                # -- 56 fp8 chunks (rotated global cols, own slab excluded) --
                pt = None
                for j in range(FP8C):
                    st_ps = ps.tile([128, 512], f32, tag="mm_ps")
                    for kp in range(2):
                        nc.tensor.matmul(
                            st_ps,
                            et8[:, 2 * kp:2 * kp + 2, j * 128:(j + 1) * 128],
                            elg8[:, 2 * kp:2 * kp + 2, r0:r0 + 512],
                            start=(kp == 0), stop=(kp == 1),
                            perf_mode=DR,
                        )
                    if j % 2 == 0:
                        pt = work.tile([128, 2, 512], e5m2, tag="p8", bufs=3)
                    nc.scalar.activation(
                        out=pt[:, j % 2:j % 2 + 1, :], in_=st_ps,
                        func=mybir.ActivationFunctionType.Exp,
                        scale=SCALE, bias=alpha_t[:, 8 + j:9 + j],
                    )
                    if j % 2 == 1:
                        for fb in range(FC):
                            nc.tensor.matmul(
                                pvt_ps[fb],
                                en8[:, j - 1:j + 1, fb * 128:(fb + 1) * 128],
                                pt,
                                start=(j == 1), stop=False,
                                perf_mode=DR,
                            )
                        nc.tensor.matmul(
                            l2_ps,
                            ones8[:, :, 0:1],
                            pt,
                            start=(j == 1), stop=False,
                            perf_mode=DR,
                        )

                # -- 8 bf16 own-slab chunks (score diagonal lives here) --
                for k in range(OWN):
                    st_ps = ps.tile([128, 512], f32, tag="mm_ps")
                    for fc in range(FC):
                        nc.tensor.matmul(
                            st_ps,
                            etl[:, fc:fc + 1, k * 128:(k + 1) * 128],
                            elg16[:, fc:fc + 1, r0:r0 + 512],
                            start=(fc == 0), stop=(fc == FC - 1),
                        )
                    ptb = work.tile([128, 512], bf16, tag="p16", bufs=3)
                    nc.scalar.activation(
                        out=ptb, in_=st_ps,
                        func=mybir.ActivationFunctionType.Exp,
                        scale=SCALE, bias=alpha_t[:, k:k + 1],
                    )
                    for fb in range(FC):
                        nc.tensor.matmul(
                            pvt_ps[fb],
                            enl[:, k:k + 1, fb * 128:(fb + 1) * 128],
                            ptb,
                            start=False, stop=(k == OWN - 1),
                        )
                    nc.tensor.matmul(
                        l2_ps,
                        ones16,
                        ptb,
                        start=False, stop=(k == OWN - 1),
                    )

                # -- epilogue: 1/L, transpose to [128,4], project, store --
                lrow = work.tile([128, 512], f32, tag="lrow", bufs=1,
                                 name=f"lrow{rb}")
                nc.vector.reciprocal(out=lrow[0:1, :], in_=l2_ps)
                lt_ps = ps.tile([128, 4], f32, tag="lT", bufs=1,
                                name=f"lt{rb}")
                for jj in range(4):
                    nc.tensor.matmul(
                        lt_ps[:, jj:jj + 1],
                        lrow[0:1, jj * 128:(jj + 1) * 128],
                        id1,
                        start=True, stop=True,
                        is_transpose=True, skip_group_check=True,
                    )
                linv = work.tile([128, 4], f32, tag="linv")
                nc.vector.tensor_copy(out=linv, in_=lt_ps)

                ptbs = [
                    work.tile([128, 512], bf16, tag="ptb", bufs=8,
                              name=f"ptb{rb}_{fb}")
                    for fb in range(FC)
                ]
                for fb in range(FC):
                    nc.scalar.activation(
                        out=ptbs[fb], in_=pvt_ps[fb],
                        func=mybir.ActivationFunctionType.Copy)
                for jj in range(4):
                    o_ps = ps.tile([128, D], f32, tag="mm_ps")
                    for fb in range(FC):
                        nc.tensor.matmul(
                            o_ps,
                            ptbs[fb][:, jj * 128:(jj + 1) * 128],
                            wv[:, fb:fb + 1, :],
                            start=(fb == 0), stop=(fb == FC - 1),
                        )
                    o_t = work.tile([128, D], f32, tag="o_t", bufs=3)
                    nc.vector.scalar_tensor_tensor(
                        out=o_t, in0=o_ps, scalar=linv[:, jj:jj + 1],
                        in1=bv_bc, op0=mybir.AluOpType.mult,
                        op1=mybir.AluOpType.add,
                    )
                    nc.sync.dma_start(
                        out=out[r0 + jj * 128: r0 + (jj + 1) * 128, :],
                        in_=o_t)

    nc.compile()
    return nc


def _get_nc():
    global _NC
    if _NC is None:
        _NC = build_kernel()
    return _NC


def kernel(embedding, W_qk, b_qk, W_v, b_v):
    global LAST_RESULT
    E = np.ascontiguousarray(np.asarray(embedding, dtype=np.float32))
    Wqk = np.asarray(W_qk, dtype=np.float32)
    bqk = np.asarray(b_qk, dtype=np.float32)
    Wv = np.asarray(W_v, dtype=np.float32)
    bv = np.ascontiguousarray(np.asarray(b_v, dtype=np.float32))

    G = Wqk.T @ Wqk
    Et = E @ G                                 # Etilde, f32
    alpha = SCALE * (E @ (Wqk.T @ bqk))        # [N] f32

    E8 = E.astype(ml_dtypes.float8_e4m3fn)
    wvT = np.ascontiguousarray(Wv.T).astype(ml_dtypes.bfloat16)

    in_maps = []
    for i in range(CORES):
        rows = np.arange(i * NL, (i + 1) * NL)
        perm = np.concatenate(
            [np.arange((i + 1) * NL, N), np.arange(0, i * NL)])
        E8p = E8[perm]
        in_maps.append({
            "et8": np.ascontiguousarray(E8p.T),
            "en8": np.ascontiguousarray(
                E8p.reshape(FP8C, 128, F).transpose(1, 0, 2)
            ).reshape(128, FP8C * F),
            "elg8": np.ascontiguousarray(Et[rows].T).astype(
                ml_dtypes.float8_e4m3fn),
            "etl": np.ascontiguousarray(E[rows].T).astype(ml_dtypes.bfloat16),
            "enl": np.ascontiguousarray(
                E[rows].astype(ml_dtypes.bfloat16).reshape(
                    OWN, 128, F).transpose(1, 0, 2)
            ).reshape(128, OWN * F),
            "elg16": np.ascontiguousarray(Et[rows].T).astype(
                ml_dtypes.bfloat16),
            "alpha_t": np.ascontiguousarray(
                np.concatenate([alpha[rows], alpha[perm]]).reshape(
                    CC, 128).T),
            "wvT": wvT,
            "bv": bv,
        })

    nc = _get_nc()
    res = run_bass_kernel_spmd(nc, in_maps, core_ids=list(range(CORES)))
    LAST_RESULT = res
    return np.concatenate(
        [np.asarray(res.results[i]["out"]) for i in range(CORES)], axis=0
    )


# revision 10
# speedup vs baseline: 1.4079x; 1.4079x over previous
"""Trainium2 8-core attention kernel v7 (N=8192, D=512, Q==K shared projection).

fp8 DoubleRow formulation on top of the projection-free algebra:

    scores = SCALE * (Etilde E^T) + alpha,   Etilde = E G,  G = W_qk^T W_qk,
    alpha  = SCALE * E (W_qk^T b_qk)         (host-precomputed, f32 exact)
    attn @ V = (P @ E) W_v^T + b_v,          P = exp(scores), row-normalized

Per core: 1024 output rows; the 64 column chunks are host-ROTATED so the
core's own slab is always chunks 0..7.  Q==K makes the score diagonal
dominate the softmax (self-scores 7..14 vs off-diag |s|<4.6), so the own
slab runs in bf16 while the other 56 chunks use fp8: e4m3 operands for the
score/PV matmuls with perf_mode=DoubleRow (2 k-subtiles per instruction),
exp output in e5m2 (max P ~66 there, well under the 57344 ceiling; the
bf16 diag path holds the exp(14.4)=1.8e6 peaks).

Row sums ride the PV matmul as a ones-column (one [1,512] PSUM
accumulator), transposed back to [128,4] via 4 tiny PE transposes —
no per-chunk DVE work at all in steady state.
"""

import ml_dtypes
import numpy as np

import concourse.bass as bass
import concourse.mybir as mybir
import concourse.tile as tile
from concourse import bacc
from concourse.bass_utils import run_bass_kernel_spmd

N = 8192          # sequence length
F = 512           # features == head dim
D = 512
CORES = 8
NL = N // CORES   # local rows per core (1024)
SCALE = 1.0 / float(np.sqrt(D))

FC = F // 128     # 4 f-chunks
CC = N // 128     # 64 column chunks
OWN = NL // 128   # 8 own (bf16) chunks
NF8 = N - NL      # 7168 fp8 columns
FP8C = NF8 // 128  # 56 fp8 chunks
RB = NL // 512    # 2 row-blocks of 512

f32 = mybir.dt.float32
bf16 = mybir.dt.bfloat16
e4m3 = mybir.dt.float8e4
e5m2 = mybir.dt.float8e5
DR = mybir.MatmulPerfMode.DoubleRow

_NC = None
LAST_RESULT = None


def build_kernel():
    nc = bacc.Bacc(target_bir_lowering=False)

    et8d = nc.declare_dram_parameter("et8", [F, NF8], e4m3, isOutput=False)
    en8d = nc.declare_dram_parameter("en8", [128, FP8C * F], e4m3, isOutput=False)
    elg8d = nc.declare_dram_parameter("elg8", [F, NL], e4m3, isOutput=False)
    etld = nc.declare_dram_parameter("etl", [F, NL], bf16, isOutput=False)
    enld = nc.declare_dram_parameter("enl", [128, OWN * F], bf16, isOutput=False)
    elg16d = nc.declare_dram_parameter("elg16", [F, NL], bf16, isOutput=False)
    alphad = nc.declare_dram_parameter("alpha_t", [128, CC], f32, isOutput=False)
    wvTd = nc.declare_dram_parameter("wvT", [F, D], bf16, isOutput=False)
    bvd = nc.declare_dram_parameter("bv", [D], f32, isOutput=False)
    out = nc.declare_dram_parameter("out", [NL, D], f32, isOutput=True)

    with tile.TileContext(nc) as tc:
        with (
            tc.tile_pool(name="persist", bufs=1) as persist,
            tc.tile_pool(name="work", bufs=2) as work,
            tc.tile_pool(name="ps", bufs=2, space="PSUM") as ps,
        ):
            # ---- startup-critical DMAs first: fp8 score operands ----
            alpha_t = persist.tile([128, CC], f32)
            nc.sync.dma_start(out=alpha_t, in_=alphad[:, :])
            elg8 = persist.tile([128, FC, NL], e4m3)
            for fc in range(FC):
                nc.sync.dma_start(
                    out=elg8[:, fc:fc + 1, 0:512],
                    in_=elg8d[fc * 128:(fc + 1) * 128, 0:512].rearrange(
                        "p (o n) -> p o n", o=1))

            # et8 / en8 streamed in slabs of 7 chunks (896 cols)
            et8 = persist.tile([128, FC, NF8], e4m3)
            en8 = persist.tile([128, FP8C, F], e4m3)
            NSLAB = 8
            SW = NF8 // NSLAB          # 896 columns per slab
            SCH = FP8C // NSLAB        # 7 chunks per slab
            for sl in range(NSLAB):
                n0 = sl * SW
                for fc in range(FC):
                    nc.sync.dma_start(
                        out=et8[:, fc:fc + 1, n0:n0 + SW],
                        in_=et8d[fc * 128:(fc + 1) * 128, n0:n0 + SW].rearrange(
                            "p (o n) -> p o n", o=1))
                nc.gpsimd.dma_start(
                    out=en8[:, sl * SCH:(sl + 1) * SCH, :],
                    in_=en8d[:, n0 * 4:(n0 + SW) * 4].rearrange(
                        "p (c f) -> p c f", f=F))
                if sl == 0:
                    for fc in range(FC):
                        nc.sync.dma_start(
                            out=elg8[:, fc:fc + 1, 512:NL],
                            in_=elg8d[fc * 128:(fc + 1) * 128, 512:NL].rearrange(
                                "p (o n) -> p o n", o=1))

            # ---- bf16 own-slab operands (consumed at the tail of each rb) ----
            etl = persist.tile([128, FC, NL], bf16)
            elg16 = persist.tile([128, FC, NL], bf16)
            enl = persist.tile([128, OWN, F], bf16)
            for fc in range(FC):
                nc.sync.dma_start(
                    out=etl[:, fc:fc + 1, :],
                    in_=etld[fc * 128:(fc + 1) * 128, :].rearrange(
                        "p (o n) -> p o n", o=1))
                nc.sync.dma_start(
                    out=elg16[:, fc:fc + 1, :],
                    in_=elg16d[fc * 128:(fc + 1) * 128, :].rearrange(
                        "p (o n) -> p o n", o=1))
            nc.gpsimd.dma_start(
                out=enl, in_=enld.rearrange("p (c f) -> p c f", f=F))

            wv = persist.tile([128, FC, D], bf16)
            for fc in range(FC):
                nc.sync.dma_start(
                    out=wv[:, fc:fc + 1, :],
                    in_=wvTd[fc * 128:(fc + 1) * 128, :].rearrange(
                        "p (o n) -> p o n", o=1))

            bv_bc = persist.tile([128, D], f32)
            bv_ap = bvd[:]
            nc.gpsimd.dma_start(out=bv_bc, in_=bass.AP(
                tensor=bv_ap.tensor, offset=bv_ap.offset,
                ap=[[0, 128], *bv_ap.ap]))

            ones16 = persist.tile([128, 1], bf16)
            nc.vector.memset(ones16, 1.0)
            ones8 = persist.tile([128, 2, 16], e4m3)
            nc.vector.memset(ones8, 1.0)
            id1 = persist.tile([1, 1], f32)
            nc.vector.memset(id1, 1.0)

            # ---- attention: 2 row-blocks of 512 local rows ----
            for rb in range(RB):
                r0 = rb * 512
                pvt_ps = [
                    ps.tile([128, 512], f32, tag="pvt_ps", bufs=4,
                            name=f"pvt{rb}_{fb}")
                    for fb in range(FC)
                ]
                l2t = ps.tile([128, 512], f32, tag="l2", bufs=1,
                              name=f"l2_{rb}")
                l2_ps = l2t[0:1, :]

                # -- 56 fp8 chunks (rotated global cols, own slab excluded) --
                # PV of pair k is emitted after the scores of pair k+1 so the
                # Exp (ScalarE) output it consumes is never on the PE critical
                # path.  Row sums accumulate on the otherwise-idle DVE.
                lacc = work.tile([128, 512], f32, tag="lacc", bufs=1,
                                 name=f"lacc{rb}")
                pt = None
                pend = None      # (pair_idx, pt_tile) awaiting PV
                first_pv = [True]

                def flush_pv(nc=nc, pvt_ps=pvt_ps):
                    pr, tile_ = pend
                    for fb in range(FC):
                        nc.tensor.matmul(
                            pvt_ps[fb],
                            en8[:, 2 * pr:2 * pr + 2, fb * 128:(fb + 1) * 128],
                            tile_,
                            start=first_pv[0], stop=False,
                            perf_mode=DR,
                        )
                    first_pv[0] = False

                for j in range(FP8C):
                    st_ps = ps.tile([128, 512], f32, tag="mm_ps", bufs=3)
                    for kp in range(2):
                        nc.tensor.matmul(
                            st_ps,
                            et8[:, 2 * kp:2 * kp + 2, j * 128:(j + 1) * 128],
                            elg8[:, 2 * kp:2 * kp + 2, r0:r0 + 512],
                            start=(kp == 0), stop=(kp == 1),
                            perf_mode=DR,
                        )
                    if j % 2 == 0:
                        pt = work.tile([128, 2, 512], e5m2, tag="p8", bufs=4)
                    nc.scalar.activation(
                        out=pt[:, j % 2:j % 2 + 1, :], in_=st_ps,
                        func=mybir.ActivationFunctionType.Exp,
                        scale=SCALE, bias=alpha_t[:, 8 + j:9 + j],
                    )
                    if j == 0:
                        nc.vector.tensor_copy(
                            out=lacc, in_=pt[:, 0:1, :])
                    else:
                        nc.vector.tensor_add(
                            lacc, lacc, pt[:, j % 2:j % 2 + 1, :])
                    if j % 2 == 1:
                        if pend is not None:
                            flush_pv()
                        pend = (j // 2, pt)
                flush_pv()

                # -- 8 bf16 own-slab chunks (score diagonal lives here) --
                pendb = None
                for k in range(OWN):
                    st_ps = ps.tile([128, 512], f32, tag="mm_ps", bufs=3)
                    for fc in range(FC):
                        nc.tensor.matmul(
                            st_ps,
                            etl[:, fc:fc + 1, k * 128:(k + 1) * 128],
                            elg16[:, fc:fc + 1, r0:r0 + 512],
                            start=(fc == 0), stop=(fc == FC - 1),
                        )
                    ptb = work.tile([128, 512], bf16, tag="p16", bufs=3)
                    nc.scalar.activation(
                        out=ptb, in_=st_ps,
                        func=mybir.ActivationFunctionType.Exp,
                        scale=SCALE, bias=alpha_t[:, k:k + 1],
                    )
                    nc.vector.tensor_add(lacc, lacc, ptb)
                    if pendb is not None:
                        kk, tile_ = pendb
                        for fb in range(FC):
                            nc.tensor.matmul(
                                pvt_ps[fb],
                                enl[:, kk:kk + 1, fb * 128:(fb + 1) * 128],
                                tile_,
                                start=False, stop=False,
                            )
                    pendb = (k, ptb)
                kk, tile_ = pendb
                for fb in range(FC):
                    nc.tensor.matmul(
                        pvt_ps[fb],
                        enl[:, kk:kk + 1, fb * 128:(fb + 1) * 128],
                        tile_,
                        start=False, stop=True,
                    )

                # -- epilogue: L row -> SBUF, transpose to [128,4], 1/L --
                lrow = work.tile([128, 512], f32, tag="lrow", bufs=1,
                                 name=f"lrow{rb}")
                nc.scalar.activation(
                    out=lrow[0:1, :], in_=l2_ps,
                    func=mybir.ActivationFunctionType.Copy)
                lt_ps = ps.tile([128, 512], f32, tag="l2", bufs=1,
                                name=f"lt{rb}")
                for jj in range(4):
                    nc.tensor.matmul(
                        lt_ps[:, jj:jj + 1],
                        lrow[0:1, jj * 128:(jj + 1) * 128],
                        id1,
                        start=True, stop=True,
                        is_transpose=True, skip_group_check=True,
                    )
                linv = work.tile([128, 4], f32, tag="linv")
                nc.vector.reciprocal(out=linv, in_=lt_ps[:, 0:4])

                ptbs = [
                    work.tile([128, 512], bf16, tag="ptb", bufs=8,
                              name=f"ptb{rb}_{fb}")
                    for fb in range(FC)
                ]
                for fb in range(FC):
                    nc.scalar.activation(
                        out=ptbs[fb], in_=pvt_ps[fb],
                        func=mybir.ActivationFunctionType.Copy)
                for jj in range(4):
                    o_ps = ps.tile([128, D], f32, tag="mm_ps", bufs=3)
                    for fb in range(FC):
                        nc.tensor.matmul(
                            o_ps,
                            ptbs[fb][:, jj * 128:(jj + 1) * 128],
                            wv[:, fb:fb + 1, :],
                            start=(fb == 0), stop=(fb == FC - 1),
                        )
                    o_t = work.tile([128, D], f32, tag="o_t", bufs=3)
                    nc.vector.scalar_tensor_tensor(
                        out=o_t, in0=o_ps, scalar=linv[:, jj:jj + 1],
                        in1=bv_bc, op0=mybir.AluOpType.mult,
                        op1=mybir.AluOpType.add,
                    )
                    nc.sync.dma_start(
                        out=out[r0 + jj * 128: r0 + (jj + 1) * 128, :],
                        in_=o_t)

    nc.compile()
    return nc


def _get_nc():
    global _NC
    if _NC is None:
        _NC = build_kernel()
    return _NC


def kernel(embedding, W_qk, b_qk, W_v, b_v):
    global LAST_RESULT
    E = np.ascontiguousarray(np.asarray(embedding, dtype=np.float32))
    Wqk = np.asarray(W_qk, dtype=np.float32)
    bqk = np.asarray(b_qk, dtype=np.float32)
    Wv = np.asarray(W_v, dtype=np.float32)
    bv = np.ascontiguousarray(np.asarray(b_v, dtype=np.float32))

    G = Wqk.T @ Wqk
    Et = E @ G                                 # Etilde, f32
    alpha = SCALE * (E @ (Wqk.T @ bqk))        # [N] f32

    E8 = E.astype(ml_dtypes.float8_e4m3fn)
    wvT = np.ascontiguousarray(Wv.T).astype(ml_dtypes.bfloat16)

    in_maps = []
    for i in range(CORES):
        rows = np.arange(i * NL, (i + 1) * NL)
        perm = np.concatenate(
            [np.arange((i + 1) * NL, N), np.arange(0, i * NL)])
        E8p = E8[perm]
        in_maps.append({
            "et8": np.ascontiguousarray(E8p.T),
            "en8": np.ascontiguousarray(
                E8p.reshape(FP8C, 128, F).transpose(1, 0, 2)
            ).reshape(128, FP8C * F),
            "elg8": np.ascontiguousarray(Et[rows].T).astype(
                ml_dtypes.float8_e4m3fn),
            "etl": np.ascontiguousarray(E[rows].T).astype(ml_dtypes.bfloat16),
            "enl": np.ascontiguousarray(
                E[rows].astype(ml_dtypes.bfloat16).reshape(
                    OWN, 128, F).transpose(1, 0, 2)
            ).reshape(128, OWN * F),
            "elg16": np.ascontiguousarray(Et[rows].T).astype(
                ml_dtypes.bfloat16),
            "alpha_t": np.ascontiguousarray(
                np.concatenate([alpha[rows], alpha[perm]]).reshape(
                    CC, 128).T),
            "wvT": wvT,
            "bv": bv,
        })

    nc = _get_nc()
    res = run_bass_kernel_spmd(nc, in_maps, core_ids=list(range(CORES)))
    LAST_RESULT = res
    return np.concatenate(
        [np.asarray(res.results[i]["out"]) for i in range(CORES)], axis=0
    )


# revision 13
# speedup vs baseline: 1.5669x; 1.1129x over previous
"""Trainium2 8-core attention kernel v7 (N=8192, D=512, Q==K shared projection).

fp8 DoubleRow formulation on top of the projection-free algebra:

    scores = SCALE * (Etilde E^T) + alpha,   Etilde = E G,  G = W_qk^T W_qk,
    alpha  = SCALE * E (W_qk^T b_qk)         (host-precomputed, f32 exact)
    attn @ V = (P @ E) W_v^T + b_v,          P = exp(scores), row-normalized

Per core: 1024 output rows; the 64 column chunks are host-ROTATED so the
core's own slab is always chunks 0..7.  Q==K makes the score diagonal
dominate the softmax (self-scores 7..14 vs off-diag |s|<4.6), so the own
slab runs in bf16 while the other 56 chunks use fp8: e4m3 operands for the
score/PV matmuls with perf_mode=DoubleRow (2 k-subtiles per instruction),
exp output in e5m2 (max P ~66 there, well under the 57344 ceiling; the
bf16 diag path holds the exp(14.4)=1.8e6 peaks).

Row sums ride the PV matmul as a ones-column (one [1,512] PSUM
accumulator), transposed back to [128,4] via 4 tiny PE transposes —
no per-chunk DVE work at all in steady state.
"""

import ml_dtypes
import numpy as np

import concourse.bass as bass
import concourse.mybir as mybir
import concourse.tile as tile
from concourse import bacc
from concourse.bass_utils import run_bass_kernel_spmd

N = 8192          # sequence length
F = 512           # features == head dim
D = 512
CORES = 8
NL = N // CORES   # local rows per core (1024)
SCALE = 1.0 / float(np.sqrt(D))

FC = F // 128     # 4 f-chunks
CC = N // 128     # 64 column chunks
OWN = NL // 128   # 8 own (bf16) chunks
NF8 = N - NL      # 7168 fp8 columns
FP8C = NF8 // 128  # 56 fp8 chunks
RB = NL // 512    # 2 row-blocks of 512

f32 = mybir.dt.float32
bf16 = mybir.dt.bfloat16
e4m3 = mybir.dt.float8e4
e5m2 = mybir.dt.float8e5
DR = mybir.MatmulPerfMode.DoubleRow

_NC = None
LAST_RESULT = None


def build_kernel():
    nc = bacc.Bacc(target_bir_lowering=False)

    et8d = nc.declare_dram_parameter("et8", [F, NF8], e4m3, isOutput=False)
    en8d = nc.declare_dram_parameter("en8", [128, FP8C * F], e4m3, isOutput=False)
    elg8d = nc.declare_dram_parameter("elg8", [F, NL], e4m3, isOutput=False)
    etld = nc.declare_dram_parameter("etl", [F, NL], bf16, isOutput=False)
    enld = nc.declare_dram_parameter("enl", [128, OWN * F], bf16, isOutput=False)
    elg16d = nc.declare_dram_parameter("elg16", [F, NL], bf16, isOutput=False)
    alphad = nc.declare_dram_parameter("alpha_t", [128, CC], f32, isOutput=False)
    wvTd = nc.declare_dram_parameter("wvT", [F, D], bf16, isOutput=False)
    bvd = nc.declare_dram_parameter("bv", [D], f32, isOutput=False)
    out = nc.declare_dram_parameter("out", [NL, D], f32, isOutput=True)

    with tile.TileContext(nc) as tc:
        with (
            tc.tile_pool(name="persist", bufs=1) as persist,
            tc.tile_pool(name="work", bufs=2) as work,
            tc.tile_pool(name="ps", bufs=2, space="PSUM") as ps,
        ):
            # ---- startup-critical DMAs first: fp8 score operands ----
            alpha_t = persist.tile([128, CC], f32)
            nc.sync.dma_start(out=alpha_t, in_=alphad[:, :])
            elg8 = persist.tile([128, FC, NL], e4m3)
            for fc in range(FC):
                nc.sync.dma_start(
                    out=elg8[:, fc:fc + 1, 0:512],
                    in_=elg8d[fc * 128:(fc + 1) * 128, 0:512].rearrange(
                        "p (o n) -> p o n", o=1))

            # et8 / en8 streamed in slabs of 7 chunks (896 cols)
            et8 = persist.tile([128, FC, NF8], e4m3)
            en8 = persist.tile([128, FP8C, F], e4m3)
            NSLAB = 8
            SW = NF8 // NSLAB          # 896 columns per slab
            SCH = FP8C // NSLAB        # 7 chunks per slab
            for sl in range(NSLAB):
                n0 = sl * SW
                for fc in range(FC):
                    nc.sync.dma_start(
                        out=et8[:, fc:fc + 1, n0:n0 + SW],
                        in_=et8d[fc * 128:(fc + 1) * 128, n0:n0 + SW].rearrange(
                            "p (o n) -> p o n", o=1))
                nc.gpsimd.dma_start(
                    out=en8[:, sl * SCH:(sl + 1) * SCH, :],
                    in_=en8d[:, n0 * 4:(n0 + SW) * 4].rearrange(
                        "p (c f) -> p c f", f=F))
                if sl == 0:
                    for fc in range(FC):
                        nc.sync.dma_start(
                            out=elg8[:, fc:fc + 1, 512:NL],
                            in_=elg8d[fc * 128:(fc + 1) * 128, 512:NL].rearrange(
                                "p (o n) -> p o n", o=1))

            # ---- bf16 own-slab operands (consumed at the tail of each rb) ----
            etl = persist.tile([128, FC, NL], bf16)
            elg16 = persist.tile([128, FC, NL], bf16)
            enl = persist.tile([128, OWN, F], bf16)
            for fc in range(FC):
                nc.sync.dma_start(
                    out=etl[:, fc:fc + 1, :],
                    in_=etld[fc * 128:(fc + 1) * 128, :].rearrange(
                        "p (o n) -> p o n", o=1))
                nc.sync.dma_start(
                    out=elg16[:, fc:fc + 1, :],
                    in_=elg16d[fc * 128:(fc + 1) * 128, :].rearrange(
                        "p (o n) -> p o n", o=1))
            nc.gpsimd.dma_start(
                out=enl, in_=enld.rearrange("p (c f) -> p c f", f=F))

            wv = persist.tile([128, FC, D], bf16)
            for fc in range(FC):
                nc.sync.dma_start(
                    out=wv[:, fc:fc + 1, :],
                    in_=wvTd[fc * 128:(fc + 1) * 128, :].rearrange(
                        "p (o n) -> p o n", o=1))

            bv_bc = persist.tile([128, D], f32)
            bv_ap = bvd[:]
            nc.gpsimd.dma_start(out=bv_bc, in_=bass.AP(
                tensor=bv_ap.tensor, offset=bv_ap.offset,
                ap=[[0, 128], *bv_ap.ap]))

            ones_f = persist.tile([128, 1], f32)
            nc.vector.memset(ones_f, 1.0)

            # ---- attention: 2 row-blocks of 512 local rows ----
            for rb in range(RB):
                r0 = rb * 512
                pvt_ps = [
                    ps.tile([128, 512], f32, tag="pvt_ps", bufs=4,
                            name=f"pvt{rb}_{fb}")
                    for fb in range(FC)
                ]


                # -- 56 fp8 chunks (rotated global cols, own slab excluded) --
                # PV of pair k is emitted after the scores of pair k+1 so the
                # Exp (ScalarE) output it consumes is never on the PE critical
                # path.  Row sums accumulate on the otherwise-idle DVE.
                lacc = work.tile([128, 512], f32, tag="lacc", bufs=1,
                                 name=f"lacc{rb}")
                pt = None
                pend = None      # (pair_idx, pt_tile) awaiting PV
                first_pv = [True]

                def flush_pv(nc=nc, pvt_ps=pvt_ps):
                    pr, tile_ = pend
                    for fb in range(FC):
                        nc.tensor.matmul(
                            pvt_ps[fb],
                            en8[:, 2 * pr:2 * pr + 2, fb * 128:(fb + 1) * 128],
                            tile_,
                            start=first_pv[0], stop=False,
                            perf_mode=DR,
                        )
                    first_pv[0] = False

                for j in range(FP8C):
                    st_ps = ps.tile([128, 512], f32, tag="mm_ps", bufs=3)
                    for kp in range(2):
                        nc.tensor.matmul(
                            st_ps,
                            et8[:, 2 * kp:2 * kp + 2, j * 128:(j + 1) * 128],
                            elg8[:, 2 * kp:2 * kp + 2, r0:r0 + 512],
                            start=(kp == 0), stop=(kp == 1),
                            perf_mode=DR,
                        )
                    if j % 2 == 0:
                        pt = work.tile([128, 2, 512], e5m2, tag="p8", bufs=4)
                    nc.scalar.activation(
                        out=pt[:, j % 2:j % 2 + 1, :], in_=st_ps,
                        func=mybir.ActivationFunctionType.Exp,
                        scale=SCALE, bias=alpha_t[:, 8 + j:9 + j],
                    )
                    if j == 0:
                        nc.vector.tensor_copy(
                            out=lacc, in_=pt[:, 0:1, :])
                    else:
                        nc.vector.tensor_add(
                            lacc, lacc, pt[:, j % 2:j % 2 + 1, :])
                    if j % 2 == 1:
                        if pend is not None:
                            flush_pv()
                        pend = (j // 2, pt)
                flush_pv()

                # -- 8 bf16 own-slab chunks (score diagonal lives here) --
                pendb = None
                for k in range(OWN):
                    st_ps = ps.tile([128, 512], f32, tag="mm_ps", bufs=3)
                    for fc in range(FC):
                        nc.tensor.matmul(
                            st_ps,
                            etl[:, fc:fc + 1, k * 128:(k + 1) * 128],
                            elg16[:, fc:fc + 1, r0:r0 + 512],
                            start=(fc == 0), stop=(fc == FC - 1),
                        )
                    ptb = work.tile([128, 512], bf16, tag="p16", bufs=3)
                    nc.scalar.activation(
                        out=ptb, in_=st_ps,
                        func=mybir.ActivationFunctionType.Exp,
                        scale=SCALE, bias=alpha_t[:, k:k + 1],
                    )
                    nc.vector.tensor_add(lacc, lacc, ptb)
                    if pendb is not None:
                        kk, tile_ = pendb
                        for fb in range(FC):
                            nc.tensor.matmul(
                                pvt_ps[fb],
                                enl[:, kk:kk + 1, fb * 128:(fb + 1) * 128],
                                tile_,
                                start=False, stop=False,
                            )
                    pendb = (k, ptb)
                kk, tile_ = pendb
                for fb in range(FC):
                    nc.tensor.matmul(
                        pvt_ps[fb],
                        enl[:, kk:kk + 1, fb * 128:(fb + 1) * 128],
                        tile_,
                        start=False, stop=True,
                    )

                # -- epilogue: L = free-reduce lacc via PE, then 1/L --
                l_ps = ps.tile([128, 4], f32, tag="l2", bufs=1,
                               name=f"l_{rb}")
                for jj in range(4):
                    nc.tensor.matmul(
                        l_ps[:, jj:jj + 1],
                        lacc[:, jj * 128:(jj + 1) * 128],
                        ones_f,
                        start=True, stop=True, skip_group_check=True,
                    )
                linv = work.tile([128, 4], f32, tag="linv")
                nc.vector.reciprocal(out=linv, in_=l_ps)

                ptbs = [
                    work.tile([128, 512], bf16, tag="ptb", bufs=8,
                              name=f"ptb{rb}_{fb}")
                    for fb in range(FC)
                ]
                for fb in range(FC):
                    nc.scalar.activation(
                        out=ptbs[fb], in_=pvt_ps[fb],
                        func=mybir.ActivationFunctionType.Copy)
                for jj in range(4):
                    o_ps = ps.tile([128, D], f32, tag="mm_ps", bufs=3)
                    for fb in range(FC):
                        nc.tensor.matmul(
                            o_ps,
                            ptbs[fb][:, jj * 128:(jj + 1) * 128],
                            wv[:, fb:fb + 1, :],
                            start=(fb == 0), stop=(fb == FC - 1),
                        )
                    o_t = work.tile([128, D], f32, tag="o_t", bufs=3)
                    nc.vector.scalar_tensor_tensor(
                        out=o_t, in0=o_ps, scalar=linv[:, jj:jj + 1],
                        in1=bv_bc, op0=mybir.AluOpType.mult,
                        op1=mybir.AluOpType.add,
                    )
                    nc.sync.dma_start(
                        out=out[r0 + jj * 128: r0 + (jj + 1) * 128, :],
                        in_=o_t)

    nc.compile()
    return nc


def _get_nc():
    global _NC
    if _NC is None:
        _NC = build_kernel()
    return _NC


def kernel(embedding, W_qk, b_qk, W_v, b_v):
    global LAST_RESULT
    E = np.ascontiguousarray(np.asarray(embedding, dtype=np.float32))
    Wqk = np.asarray(W_qk, dtype=np.float32)
    bqk = np.asarray(b_qk, dtype=np.float32)
    Wv = np.asarray(W_v, dtype=np.float32)
    bv = np.ascontiguousarray(np.asarray(b_v, dtype=np.float32))

    G = Wqk.T @ Wqk
    Et = E @ G                                 # Etilde, f32
    alpha = SCALE * (E @ (Wqk.T @ bqk))        # [N] f32

    E8 = E.astype(ml_dtypes.float8_e4m3fn)
    wvT = np.ascontiguousarray(Wv.T).astype(ml_dtypes.bfloat16)

    in_maps = []
    for i in range(CORES):
        rows = np.arange(i * NL, (i + 1) * NL)
        perm = np.concatenate(
            [np.arange((i + 1) * NL, N), np.arange(0, i * NL)])
        E8p = E8[perm]
        in_maps.append({
            "et8": np.ascontiguousarray(E8p.T),
            "en8": np.ascontiguousarray(
                E8p.reshape(FP8C, 128, F).transpose(1, 0, 2)
            ).reshape(128, FP8C * F),
            "elg8": np.ascontiguousarray(Et[rows].T).astype(
                ml_dtypes.float8_e4m3fn),
            "etl": np.ascontiguousarray(E[rows].T).astype(ml_dtypes.bfloat16),
            "enl": np.ascontiguousarray(
                E[rows].astype(ml_dtypes.bfloat16).reshape(
                    OWN, 128, F).transpose(1, 0, 2)
            ).reshape(128, OWN * F),
            "elg16": np.ascontiguousarray(Et[rows].T).astype(
                ml_dtypes.bfloat16),
            "alpha_t": np.ascontiguousarray(
                np.concatenate([alpha[rows], alpha[perm]]).reshape(
                    CC, 128).T),
            "wvT": wvT,
            "bv": bv,
        })

    nc = _get_nc()
    res = run_bass_kernel_spmd(nc, in_maps, core_ids=list(range(CORES)))
    LAST_RESULT = res
    return np.concatenate(
        [np.asarray(res.results[i]["out"]) for i in range(CORES)], axis=0
    )


# revision 14
# speedup vs baseline: 1.6065x; 1.0252x over previous
"""Trainium2 8-core attention kernel v7 (N=8192, D=512, Q==K shared projection).

fp8 DoubleRow formulation on top of the projection-free algebra:

    scores = SCALE * (Etilde E^T) + alpha,   Etilde = E G,  G = W_qk^T W_qk,
    alpha  = SCALE * E (W_qk^T b_qk)         (host-precomputed, f32 exact)
    attn @ V = (P @ E) W_v^T + b_v,          P = exp(scores), row-normalized

Per core: 1024 output rows; the 64 column chunks are host-ROTATED so the
core's own slab is always chunks 0..7.  Q==K makes the score diagonal
dominate the softmax (self-scores 7..14 vs off-diag |s|<4.6), so the own
slab runs in bf16 while the other 56 chunks use fp8: e4m3 operands for the
score/PV matmuls with perf_mode=DoubleRow (2 k-subtiles per instruction),
exp output in e5m2 (max P ~66 there, well under the 57344 ceiling; the
bf16 diag path holds the exp(14.4)=1.8e6 peaks).

Row sums ride the PV matmul as a ones-column (one [1,512] PSUM
accumulator), transposed back to [128,4] via 4 tiny PE transposes —
no per-chunk DVE work at all in steady state.
"""

import ml_dtypes
import numpy as np

import concourse.bass as bass
import concourse.mybir as mybir
import concourse.tile as tile
from concourse import bacc
from concourse.bass_utils import run_bass_kernel_spmd

N = 8192          # sequence length
F = 512           # features == head dim
D = 512
CORES = 8
NL = N // CORES   # local rows per core (1024)
SCALE = 1.0 / float(np.sqrt(D))

FC = F // 128     # 4 f-chunks
CC = N // 128     # 64 column chunks
OWN = NL // 128   # 8 own (bf16) chunks
NF8 = N - NL      # 7168 fp8 columns
FP8C = NF8 // 128  # 56 fp8 chunks
RB = NL // 512    # 2 row-blocks of 512

f32 = mybir.dt.float32
bf16 = mybir.dt.bfloat16
e4m3 = mybir.dt.float8e4
e5m2 = mybir.dt.float8e5
DR = mybir.MatmulPerfMode.DoubleRow

_NC = None
LAST_RESULT = None


def build_kernel():
    nc = bacc.Bacc(target_bir_lowering=False)

    et8d = nc.declare_dram_parameter("et8", [F, NF8], e4m3, isOutput=False)
    en8d = nc.declare_dram_parameter("en8", [128, FP8C * F], e4m3, isOutput=False)
    elg8d = nc.declare_dram_parameter("elg8", [F, NL], e4m3, isOutput=False)
    etld = nc.declare_dram_parameter("etl", [F, NL], bf16, isOutput=False)
    enld = nc.declare_dram_parameter("enl", [128, OWN * F], bf16, isOutput=False)
    elg16d = nc.declare_dram_parameter("elg16", [F, NL], bf16, isOutput=False)
    alphad = nc.declare_dram_parameter("alpha_t", [128, CC], f32, isOutput=False)
    wvTd = nc.declare_dram_parameter("wvT", [F, D], bf16, isOutput=False)
    bvd = nc.declare_dram_parameter("bv", [D], f32, isOutput=False)
    out = nc.declare_dram_parameter("out", [NL, D], f32, isOutput=True)

    with tile.TileContext(nc) as tc:
        with (
            tc.tile_pool(name="persist", bufs=1) as persist,
            tc.tile_pool(name="work", bufs=2) as work,
            tc.tile_pool(name="ps", bufs=2, space="PSUM") as ps,
        ):
            # ---- startup-critical DMAs first: fp8 score operands ----
            alpha_t = persist.tile([128, CC], f32)
            nc.sync.dma_start(out=alpha_t, in_=alphad[:, :])
            elg8 = persist.tile([128, FC, NL], e4m3)
            nc.sync.dma_start(
                out=elg8[:, :, 0:512],
                in_=elg8d[:, 0:512].rearrange("(f p) n -> p f n", p=128))

            # et8 / en8 streamed in slabs of 7 chunks (896 cols)
            et8 = persist.tile([128, FC, NF8], e4m3)
            en8 = persist.tile([128, FP8C, F], e4m3)
            NSLAB = 8
            SW = NF8 // NSLAB          # 896 columns per slab
            SCH = FP8C // NSLAB        # 7 chunks per slab
            for sl in range(NSLAB):
                n0 = sl * SW
                nc.sync.dma_start(
                    out=et8[:, :, n0:n0 + SW],
                    in_=et8d[:, n0:n0 + SW].rearrange("(f p) n -> p f n", p=128))
                nc.gpsimd.dma_start(
                    out=en8[:, sl * SCH:(sl + 1) * SCH, :],
                    in_=en8d[:, n0 * 4:(n0 + SW) * 4].rearrange(
                        "p (c f) -> p c f", f=F))
                if sl == 0:
                    nc.sync.dma_start(
                        out=elg8[:, :, 512:NL],
                        in_=elg8d[:, 512:NL].rearrange(
                            "(f p) n -> p f n", p=128))

            # ---- bf16 own-slab operands (consumed at the tail of each rb) ----
            etl = persist.tile([128, FC, NL], bf16)
            elg16 = persist.tile([128, FC, NL], bf16)
            enl = persist.tile([128, OWN, F], bf16)
            nc.sync.dma_start(
                out=etl, in_=etld[:, :].rearrange("(f p) n -> p f n", p=128))
            nc.sync.dma_start(
                out=elg16, in_=elg16d[:, :].rearrange("(f p) n -> p f n", p=128))
            nc.gpsimd.dma_start(
                out=enl, in_=enld.rearrange("p (c f) -> p c f", f=F))

            wv = persist.tile([128, FC, D], bf16)
            nc.sync.dma_start(
                out=wv, in_=wvTd[:, :].rearrange("(f p) d -> p f d", p=128))

            bv_bc = persist.tile([128, D], f32)
            bv_ap = bvd[:]
            nc.gpsimd.dma_start(out=bv_bc, in_=bass.AP(
                tensor=bv_ap.tensor, offset=bv_ap.offset,
                ap=[[0, 128], *bv_ap.ap]))

            ones_f = persist.tile([128, 1], f32)
            nc.vector.memset(ones_f, 1.0)

            # ---- attention: 2 row-blocks of 512 local rows ----
            for rb in range(RB):
                r0 = rb * 512
                pvt_ps = [
                    ps.tile([128, 512], f32, tag="pvt_ps", bufs=4,
                            name=f"pvt{rb}_{fb}")
                    for fb in range(FC)
                ]


                # -- 56 fp8 chunks (rotated global cols, own slab excluded) --
                # PV of pair k is emitted after the scores of pair k+1 so the
                # Exp (ScalarE) output it consumes is never on the PE critical
                # path.  Row sums accumulate on the otherwise-idle DVE.
                lacc = work.tile([128, 512], f32, tag="lacc", bufs=1,
                                 name=f"lacc{rb}")
                pt = None
                pend = None      # (pair_idx, pt_tile) awaiting PV
                first_pv = [True]

                def flush_pv(nc=nc, pvt_ps=pvt_ps):
                    pr, tile_ = pend
                    for fb in range(FC):
                        nc.tensor.matmul(
                            pvt_ps[fb],
                            en8[:, 2 * pr:2 * pr + 2, fb * 128:(fb + 1) * 128],
                            tile_,
                            start=first_pv[0], stop=False,
                            perf_mode=DR,
                        )
                    first_pv[0] = False

                for j in range(FP8C):
                    st_ps = ps.tile([128, 512], f32, tag="mm_ps", bufs=3)
                    for kp in range(2):
                        nc.tensor.matmul(
                            st_ps,
                            et8[:, 2 * kp:2 * kp + 2, j * 128:(j + 1) * 128],
                            elg8[:, 2 * kp:2 * kp + 2, r0:r0 + 512],
                            start=(kp == 0), stop=(kp == 1),
                            perf_mode=DR,
                        )
                    if j % 2 == 0:
                        pt = work.tile([128, 2, 512], e5m2, tag="p8", bufs=4)
                    nc.scalar.activation(
                        out=pt[:, j % 2:j % 2 + 1, :], in_=st_ps,
                        func=mybir.ActivationFunctionType.Exp,
                        scale=SCALE, bias=alpha_t[:, 8 + j:9 + j],
                    )
                    if j == 0:
                        nc.vector.tensor_copy(
                            out=lacc, in_=pt[:, 0:1, :])
                    else:
                        nc.vector.tensor_add(
                            lacc, lacc, pt[:, j % 2:j % 2 + 1, :])
                    if j % 2 == 1:
                        if pend is not None:
                            flush_pv()
                        pend = (j // 2, pt)
                flush_pv()

                # -- 8 bf16 own-slab chunks (score diagonal lives here) --
                pendb = None
                for k in range(OWN):
                    st_ps = ps.tile([128, 512], f32, tag="mm_ps", bufs=3)
                    for fc in range(FC):
                        nc.tensor.matmul(
                            st_ps,
                            etl[:, fc:fc + 1, k * 128:(k + 1) * 128],
                            elg16[:, fc:fc + 1, r0:r0 + 512],
                            start=(fc == 0), stop=(fc == FC - 1),
                        )
                    ptb = work.tile([128, 512], bf16, tag="p16", bufs=3)
                    nc.scalar.activation(
                        out=ptb, in_=st_ps,
                        func=mybir.ActivationFunctionType.Exp,
                        scale=SCALE, bias=alpha_t[:, k:k + 1],
                    )
                    nc.vector.tensor_add(lacc, lacc, ptb)
                    if pendb is not None:
                        kk, tile_ = pendb
                        for fb in range(FC):
                            nc.tensor.matmul(
                                pvt_ps[fb],
                                enl[:, kk:kk + 1, fb * 128:(fb + 1) * 128],
                                tile_,
                                start=False, stop=False,
                            )
                    pendb = (k, ptb)
                kk, tile_ = pendb
                for fb in range(FC):
                    nc.tensor.matmul(
                        pvt_ps[fb],
                        enl[:, kk:kk + 1, fb * 128:(fb + 1) * 128],
                        tile_,
                        start=False, stop=True,
                    )

                # -- epilogue: L = free-reduce lacc via PE, then 1/L --
                l_ps = ps.tile([128, 4], f32, tag="l2", bufs=1,
                               name=f"l_{rb}")
                for jj in range(4):
                    nc.tensor.matmul(
                        l_ps[:, jj:jj + 1],
                        lacc[:, jj * 128:(jj + 1) * 128],
                        ones_f,
                        start=True, stop=True, skip_group_check=True,
                    )
                linv = work.tile([128, 4], f32, tag="linv")
                nc.vector.reciprocal(out=linv, in_=l_ps)

                ptbs = [
                    work.tile([128, 512], bf16, tag="ptb", bufs=8,
                              name=f"ptb{rb}_{fb}")
                    for fb in range(FC)
                ]
                for fb in range(FC):
                    nc.scalar.activation(
                        out=ptbs[fb], in_=pvt_ps[fb],
                        func=mybir.ActivationFunctionType.Copy)
                for jj in range(4):
                    o_ps = ps.tile([128, D], f32, tag="mm_ps", bufs=3)
                    for fb in range(FC):
                        nc.tensor.matmul(
                            o_ps,
                            ptbs[fb][:, jj * 128:(jj + 1) * 128],
                            wv[:, fb:fb + 1, :],
                            start=(fb == 0), stop=(fb == FC - 1),
                        )
                    o_t = work.tile([128, D], f32, tag="o_t", bufs=3)
                    nc.vector.scalar_tensor_tensor(
                        out=o_t, in0=o_ps, scalar=linv[:, jj:jj + 1],
                        in1=bv_bc, op0=mybir.AluOpType.mult,
                        op1=mybir.AluOpType.add,
                    )
                    nc.sync.dma_start(
                        out=out[r0 + jj * 128: r0 + (jj + 1) * 128, :],
                        in_=o_t)

    nc.compile()
    return nc


def _get_nc():
    global _NC
    if _NC is None:
        _NC = build_kernel()
    return _NC


def kernel(embedding, W_qk, b_qk, W_v, b_v):
    global LAST_RESULT
    E = np.ascontiguousarray(np.asarray(embedding, dtype=np.float32))
    Wqk = np.asarray(W_qk, dtype=np.float32)
    bqk = np.asarray(b_qk, dtype=np.float32)
    Wv = np.asarray(W_v, dtype=np.float32)
    bv = np.ascontiguousarray(np.asarray(b_v, dtype=np.float32))

    G = Wqk.T @ Wqk
    Et = E @ G                                 # Etilde, f32
    alpha = SCALE * (E @ (Wqk.T @ bqk))        # [N] f32

    E8 = E.astype(ml_dtypes.float8_e4m3fn)
    wvT = np.ascontiguousarray(Wv.T).astype(ml_dtypes.bfloat16)

    in_maps = []
    for i in range(CORES):
        rows = np.arange(i * NL, (i + 1) * NL)
        perm = np.concatenate(
            [np.arange((i + 1) * NL, N), np.arange(0, i * NL)])
        E8p = E8[perm]
        in_maps.append({
            "et8": np.ascontiguousarray(E8p.T),
            "en8": np.ascontiguousarray(
                E8p.reshape(FP8C, 128, F).transpose(1, 0, 2)
            ).reshape(128, FP8C * F),
            "elg8": np.ascontiguousarray(Et[rows].T).astype(
                ml_dtypes.float8_e4m3fn),
            "etl": np.ascontiguousarray(E[rows].T).astype(ml_dtypes.bfloat16),
            "enl": np.ascontiguousarray(
                E[rows].astype(ml_dtypes.bfloat16).reshape(
                    OWN, 128, F).transpose(1, 0, 2)
            ).reshape(128, OWN * F),
            "elg16": np.ascontiguousarray(Et[rows].T).astype(
                ml_dtypes.bfloat16),
            "alpha_t": np.ascontiguousarray(
                np.concatenate([alpha[rows], alpha[perm]]).reshape(
                    CC, 128).T),
            "wvT": wvT,
            "bv": bv,
        })

    nc = _get_nc()
    res = run_bass_kernel_spmd(nc, in_maps, core_ids=list(range(CORES)))
    LAST_RESULT = res
    return np.concatenate(
        [np.asarray(res.results[i]["out"]) for i in range(CORES)], axis=0
    )


# revision 16
# speedup vs baseline: 1.6155x; 1.0056x over previous
"""Trainium2 8-core attention kernel v8 (N=8192, D=512, Q==K shared projection).

fp8 DoubleRow formulation on top of the projection-free algebra:

    scores = SCALE * (Etilde E^T) + alpha,   Etilde = E G,  G = W_qk^T W_qk,
    alpha  = SCALE * E (W_qk^T b_qk)         (host-precomputed, f32 exact)
    attn @ V,  V = E W_v^T                   (host-precomputed; b_v added
                                              after row-normalization)

Per core: 1024 output rows; the 64 column chunks are host-ROTATED so the
core's own slab is chunks 0..7.  Q==K makes the score diagonal dominate
the softmax (self-scores 7..14.4 vs off-diag |s|<4.6), so per row-block
only the 4 chunks holding that block's diagonal run in bf16; the other 60
use fp8: e4m3 operands with perf_mode=DoubleRow (2 k-subtiles per
instruction ~2x PE rate), exp output in e5m2 (fp8 P <= ~66, ceiling 57344;
the bf16 diag path holds the exp(14.4)=1.8e6 peaks).

PV(pair k) is emitted after the scores of pair k+1 so the ScalarE exp it
consumes is off the PE critical path.  Row sums accumulate on the idle
DVE; (P V)^T comes back to row-major through 16 tiny PE transposes.
Startup-critical DMA descriptors are split across five engine queues so
the first score tile lands ~12us in.
"""

import ml_dtypes
import numpy as np

import concourse.bass as bass
import concourse.mybir as mybir
import concourse.tile as tile
from concourse import bacc
from concourse.bass_utils import run_bass_kernel_spmd

N = 8192          # sequence length
F = 512           # features == head dim
D = 512
CORES = 8
NL = N // CORES   # local rows per core (1024)
SCALE = 1.0 / float(np.sqrt(D))

FC = F // 128     # 4 f-chunks
CC = N // 128     # 64 column chunks
OWN = NL // 128   # 8 own-slab chunks
RB = NL // 512    # 2 row-blocks of 512
BCH = 4           # bf16 chunks per row-block (the diag band)

f32 = mybir.dt.float32
bf16 = mybir.dt.bfloat16
e4m3 = mybir.dt.float8e4
e5m2 = mybir.dt.float8e5
DR = mybir.MatmulPerfMode.DoubleRow

_NC = None
LAST_RESULT = None


def build_kernel():
    nc = bacc.Bacc(target_bir_lowering=False)

    et8d = nc.declare_dram_parameter("et8", [F, N], e4m3, isOutput=False)
    vn8d = nc.declare_dram_parameter("vn8", [128, CC * F], e4m3, isOutput=False)
    elg8d = nc.declare_dram_parameter("elg8", [F, NL], e4m3, isOutput=False)
    etld = nc.declare_dram_parameter("etl", [F, NL], bf16, isOutput=False)
    vnld = nc.declare_dram_parameter("vnl", [128, OWN * F], bf16, isOutput=False)
    elg16d = nc.declare_dram_parameter("elg16", [F, NL], bf16, isOutput=False)
    alphad = nc.declare_dram_parameter("alpha_t", [128, CC], f32, isOutput=False)
    identd = nc.declare_dram_parameter("ident", [128, 128], bf16, isOutput=False)
    bvd = nc.declare_dram_parameter("bv", [D], f32, isOutput=False)
    out = nc.declare_dram_parameter("out", [NL, D], f32, isOutput=True)

    with tile.TileContext(nc) as tc:
        with (
            tc.tile_pool(name="persist", bufs=1) as persist,
            tc.tile_pool(name="work", bufs=2) as work,
            tc.tile_pool(name="ps", bufs=2, space="PSUM") as ps,
        ):
            # ---- startup-critical DMAs, spread over engine queues ----
            alpha_t = persist.tile([128, CC], f32)
            et8 = persist.tile([128, FC, N], e4m3)
            vn8 = persist.tile([128, CC, F], e4m3)
            elg8 = persist.tile([128, FC, NL], e4m3)

            nc.scalar.dma_start(out=alpha_t, in_=alphad[:, :])
            for h in range(2):
                nc.scalar.dma_start(
                    out=elg8[:, :, h * 256:(h + 1) * 256],
                    in_=elg8d[:, h * 256:(h + 1) * 256].rearrange(
                        "(f p) n -> p f n", p=128))
            for q in range(4):
                nc.sync.dma_start(
                    out=et8[:, :, q * 256:(q + 1) * 256],
                    in_=et8d[:, q * 256:(q + 1) * 256].rearrange(
                        "(f p) n -> p f n", p=128))
            for h in range(2):
                nc.gpsimd.dma_start(
                    out=vn8[:, h * 4:(h + 1) * 4, :],
                    in_=vn8d[:, h * 4 * F:(h + 1) * 4 * F].rearrange(
                        "p (c f) -> p c f", f=F))

            # remaining streams (slabs of 8 chunks)
            nc.scalar.dma_start(
                out=elg8[:, :, 512:NL],
                in_=elg8d[:, 512:NL].rearrange("(f p) n -> p f n", p=128))
            for sl in range(1, 8):
                n0 = sl * 1024
                nc.sync.dma_start(
                    out=et8[:, :, n0:n0 + 1024],
                    in_=et8d[:, n0:n0 + 1024].rearrange(
                        "(f p) n -> p f n", p=128))
                nc.gpsimd.dma_start(
                    out=vn8[:, sl * 8:(sl + 1) * 8, :],
                    in_=vn8d[:, sl * 8 * F:(sl + 1) * 8 * F].rearrange(
                        "p (c f) -> p c f", f=F))

            # ---- bf16 diag-band operands (tail of each rb) ----
            etl = persist.tile([128, FC, NL], bf16)
            elg16 = persist.tile([128, FC, NL], bf16)
            vnl = persist.tile([128, OWN, F], bf16)
            nc.sync.dma_start(
                out=etl, in_=etld[:, :].rearrange("(f p) n -> p f n", p=128))
            nc.sync.dma_start(
                out=elg16, in_=elg16d[:, :].rearrange("(f p) n -> p f n", p=128))
            nc.gpsimd.dma_start(
                out=vnl, in_=vnld.rearrange("p (c f) -> p c f", f=F))

            ident = persist.tile([128, 128], bf16)
            nc.scalar.dma_start(out=ident, in_=identd[:, :])
            bv_bc = persist.tile([128, D], f32)
            bv_ap = bvd[:]
            nc.gpsimd.dma_start(out=bv_bc, in_=bass.AP(
                tensor=bv_ap.tensor, offset=bv_ap.offset,
                ap=[[0, 128], *bv_ap.ap]))

            ones_f = persist.tile([128, 1], f32)
            nc.vector.memset(ones_f, 1.0)

            # ---- attention: 2 row-blocks of 512 local rows ----
            for rb in range(RB):
                r0 = rb * 512
                b0 = rb * BCH                   # bf16 band: chunks b0..b0+3
                ids = [c for c in range(CC) if not b0 <= c < b0 + BCH]
                pvt_ps = [
                    ps.tile([128, 512], f32, tag="pvt_ps", bufs=4,
                            name=f"pvt{rb}_{fb}")
                    for fb in range(FC)
                ]
                lacc = work.tile([128, 512], f32, tag="lacc", bufs=1,
                                 name=f"lacc{rb}")
                pt = None
                pend = None      # (pair_base_chunk, pt_tile) awaiting PV
                first_pv = [True]

                def flush_pv(nc=nc, pvt_ps=pvt_ps):
                    cp, tile_ = pend
                    for fb in range(FC):
                        nc.tensor.matmul(
                            pvt_ps[fb],
                            vn8[:, cp:cp + 2, fb * 128:(fb + 1) * 128],
                            tile_,
                            start=first_pv[0], stop=False,
                            perf_mode=DR,
                        )
                    first_pv[0] = False

                for t, c in enumerate(ids):
                    st_ps = ps.tile([128, 512], f32, tag="mm_ps", bufs=3)
                    for kp in range(2):
                        nc.tensor.matmul(
                            st_ps,
                            et8[:, 2 * kp:2 * kp + 2, c * 128:(c + 1) * 128],
                            elg8[:, 2 * kp:2 * kp + 2, r0:r0 + 512],
                            start=(kp == 0), stop=(kp == 1),
                            perf_mode=DR,
                        )
                    if t % 2 == 0:
                        pt = work.tile([128, 2, 512], e5m2, tag="p8", bufs=4)
                    nc.scalar.activation(
                        out=pt[:, t % 2:t % 2 + 1, :], in_=st_ps,
                        func=mybir.ActivationFunctionType.Exp,
                        scale=SCALE, bias=alpha_t[:, c:c + 1],
                    )
                    if t == 0:
                        nc.vector.tensor_copy(out=lacc, in_=pt[:, 0:1, :])
                    else:
                        nc.vector.tensor_add(
                            lacc, lacc, pt[:, t % 2:t % 2 + 1, :])
                    if t % 2 == 1:
                        if pend is not None:
                            flush_pv()
                        pend = (c - 1, pt)
                flush_pv()

                # -- 4 bf16 chunks: this row-block's diagonal band --
                pendb = None
                for k in range(b0, b0 + BCH):
                    st_ps = ps.tile([128, 512], f32, tag="mm_ps", bufs=3)
                    for fc in range(FC):
                        nc.tensor.matmul(
                            st_ps,
                            etl[:, fc:fc + 1, k * 128:(k + 1) * 128],
                            elg16[:, fc:fc + 1, r0:r0 + 512],
                            start=(fc == 0), stop=(fc == FC - 1),
                        )
                    ptb = work.tile([128, 512], bf16, tag="p16", bufs=3)
                    nc.scalar.activation(
                        out=ptb, in_=st_ps,
                        func=mybir.ActivationFunctionType.Exp,
                        scale=SCALE, bias=alpha_t[:, k:k + 1],
                    )
                    nc.vector.tensor_add(lacc, lacc, ptb)
                    if pendb is not None:
                        kk, tile_ = pendb
                        for fb in range(FC):
                            nc.tensor.matmul(
                                pvt_ps[fb],
                                vnl[:, kk:kk + 1, fb * 128:(fb + 1) * 128],
                                tile_,
                                start=False, stop=False,
                            )
                    pendb = (k, ptb)
                kk, tile_ = pendb
                for fb in range(FC):
                    nc.tensor.matmul(
                        pvt_ps[fb],
                        vnl[:, kk:kk + 1, fb * 128:(fb + 1) * 128],
                        tile_,
                        start=False, stop=True,
                    )

                # -- epilogue: L, 1/L, transpose (P V)^T, scale+bias, store --
                l_ps = ps.tile([128, 4], f32, tag="l2", bufs=1,
                               name=f"l_{rb}")
                for jj in range(4):
                    nc.tensor.matmul(
                        l_ps[:, jj:jj + 1],
                        lacc[:, jj * 128:(jj + 1) * 128],
                        ones_f,
                        start=True, stop=True, skip_group_check=True,
                    )
                linv = work.tile([128, 4], f32, tag="linv")
                nc.vector.reciprocal(out=linv, in_=l_ps)

                ptbs = [
                    work.tile([128, 512], bf16, tag="ptb", bufs=8,
                              name=f"ptb{rb}_{fb}")
                    for fb in range(FC)
                ]
                for fb in range(FC):
                    nc.scalar.activation(
                        out=ptbs[fb], in_=pvt_ps[fb],
                        func=mybir.ActivationFunctionType.Copy)
                for jj in range(4):
                    tp_ps = ps.tile([128, 512], bf16, tag="mm_ps", bufs=3,
                                    name=f"tp{rb}_{jj}")
                    for fb in range(FC):
                        nc.tensor.matmul(
                            tp_ps[:, fb * 128:(fb + 1) * 128],
                            ptbs[fb][:, jj * 128:(jj + 1) * 128],
                            ident,
                            start=True, stop=True,
                            is_transpose=True, skip_group_check=True,
                        )
                    o_t = work.tile([128, D], f32, tag="o_t", bufs=3)
                    nc.vector.scalar_tensor_tensor(
                        out=o_t, in0=tp_ps, scalar=linv[:, jj:jj + 1],
                        in1=bv_bc, op0=mybir.AluOpType.mult,
                        op1=mybir.AluOpType.add,
                    )
                    nc.sync.dma_start(
                        out=out[r0 + jj * 128: r0 + (jj + 1) * 128, :],
                        in_=o_t)

    nc.compile()
    return nc


def _get_nc():
    global _NC
    if _NC is None:
        _NC = build_kernel()
    return _NC


def kernel(embedding, W_qk, b_qk, W_v, b_v):
    global LAST_RESULT
    E = np.ascontiguousarray(np.asarray(embedding, dtype=np.float32))
    Wqk = np.asarray(W_qk, dtype=np.float32)
    bqk = np.asarray(b_qk, dtype=np.float32)
    Wv = np.asarray(W_v, dtype=np.float32)
    bv = np.ascontiguousarray(np.asarray(b_v, dtype=np.float32))

    G = Wqk.T @ Wqk
    Et = E @ G                                 # Etilde, f32
    alpha = SCALE * (E @ (Wqk.T @ bqk))        # [N] f32
    V = E @ Wv.T                               # f32 (b_v added on device)

    E8 = E.astype(ml_dtypes.float8_e4m3fn)
    V8 = V.astype(ml_dtypes.float8_e4m3fn)
    ident = np.eye(128, dtype=ml_dtypes.bfloat16)

    in_maps = []
    for i in range(CORES):
        rows = np.arange(i * NL, (i + 1) * NL)
        perm = np.concatenate([rows, np.arange((i + 1) * NL, N),
                               np.arange(0, i * NL)])
        in_maps.append({
            "et8": np.ascontiguousarray(E8[perm].T),
            "vn8": np.ascontiguousarray(
                V8[perm].reshape(CC, 128, F).transpose(1, 0, 2)
            ).reshape(128, CC * F),
            "elg8": np.ascontiguousarray(Et[rows].T).astype(
                ml_dtypes.float8_e4m3fn),
            "etl": np.ascontiguousarray(E[rows].T).astype(ml_dtypes.bfloat16),
            "vnl": np.ascontiguousarray(
                V[rows].astype(ml_dtypes.bfloat16).reshape(
                    OWN, 128, F).transpose(1, 0, 2)
            ).reshape(128, OWN * F),
            "elg16": np.ascontiguousarray(Et[rows].T).astype(
                ml_dtypes.bfloat16),
            "alpha_t": np.ascontiguousarray(
                alpha[perm].reshape(CC, 128).T),
            "ident": ident,
            "bv": bv,
        })

    nc = _get_nc()
    res = run_bass_kernel_spmd(nc, in_maps, core_ids=list(range(CORES)))
    LAST_RESULT = res
    return np.concatenate(
        [np.asarray(res.results[i]["out"]) for i in range(CORES)], axis=0
    )


# revision 20
# speedup vs baseline: 1.6397x; 1.0150x over previous
"""Trainium2 8-core attention kernel v8 (N=8192, D=512, Q==K shared projection).

fp8 DoubleRow formulation on top of the projection-free algebra:

    scores = SCALE * (Etilde E^T) + alpha,   Etilde = E G,  G = W_qk^T W_qk,
    alpha  = SCALE * E (W_qk^T b_qk)         (host-precomputed, f32 exact)
    attn @ V,  V = E W_v^T                   (host-precomputed; b_v added
                                              after row-normalization)

Per core: 1024 output rows; the 64 column chunks are host-ROTATED so the
core's own slab is chunks 0..7.  Q==K makes the score diagonal dominate
the softmax (self-scores 7..14.4 vs off-diag |s|<4.6), so per row-block
only the 4 chunks holding that block's diagonal run in bf16; the other 60
use fp8: e4m3 operands with perf_mode=DoubleRow (2 k-subtiles per
instruction ~2x PE rate), exp output in e5m2 (fp8 P <= ~66, ceiling 57344;
the bf16 diag path holds the exp(14.4)=1.8e6 peaks).

PV(pair k) is emitted after the scores of pair k+1 so the ScalarE exp it
consumes is off the PE critical path.  Row sums accumulate on the idle
DVE; (P V)^T comes back to row-major through 16 tiny PE transposes.
Startup-critical DMA descriptors are split across five engine queues so
the first score tile lands ~12us in.
"""

import ml_dtypes
import numpy as np

import concourse.bass as bass
import concourse.mybir as mybir
import concourse.tile as tile
from concourse import bacc
from concourse.bass_utils import run_bass_kernel_spmd
from concourse.tile_rust import add_dep_helper

N = 8192          # sequence length
F = 512           # features == head dim
D = 512
CORES = 8
NL = N // CORES   # local rows per core (1024)
SCALE = 1.0 / float(np.sqrt(D))

FC = F // 128     # 4 f-chunks
CC = N // 128     # 64 column chunks
OWN = NL // 128   # 8 own-slab chunks
RB = NL // 512    # 2 row-blocks of 512
BCH = 4           # bf16 chunks per row-block (the diag band)

f32 = mybir.dt.float32
bf16 = mybir.dt.bfloat16
e4m3 = mybir.dt.float8e4
e5m2 = mybir.dt.float8e5
DR = mybir.MatmulPerfMode.DoubleRow

_NC = None
LAST_RESULT = None


def build_kernel():
    nc = bacc.Bacc(target_bir_lowering=False)

    et8d = nc.declare_dram_parameter("et8", [F, N], e4m3, isOutput=False)
    vn8d = nc.declare_dram_parameter("vn8", [128, CC * F], e4m3, isOutput=False)
    elg8d = nc.declare_dram_parameter("elg8", [F, NL], e4m3, isOutput=False)
    etld = nc.declare_dram_parameter("etl", [F, NL], bf16, isOutput=False)
    vnld = nc.declare_dram_parameter("vnl", [128, OWN * F], bf16, isOutput=False)
    elg16d = nc.declare_dram_parameter("elg16", [F, NL], bf16, isOutput=False)
    alphad = nc.declare_dram_parameter("alpha_t", [128, CC], f32, isOutput=False)
    identd = nc.declare_dram_parameter("ident", [128, 128], bf16, isOutput=False)
    bvd = nc.declare_dram_parameter("bv", [D], f32, isOutput=False)
    out = nc.declare_dram_parameter("out", [NL, D], f32, isOutput=True)

    with tile.TileContext(nc) as tc:
        with (
            tc.tile_pool(name="persist", bufs=1) as persist,
            tc.tile_pool(name="work", bufs=2) as work,
            tc.tile_pool(name="ps", bufs=2, space="PSUM") as ps,
        ):
            # ---- startup-critical DMAs, spread over engine queues ----
            alpha_t = persist.tile([128, CC], f32)
            et8 = persist.tile([128, FC, N], e4m3)
            vn8 = persist.tile([128, CC, F], e4m3)
            elg8 = persist.tile([128, FC, NL], e4m3)

            nc.scalar.dma_start(out=alpha_t, in_=alphad[:, :])
            for h in range(2):
                nc.scalar.dma_start(
                    out=elg8[:, :, h * 256:(h + 1) * 256],
                    in_=elg8d[:, h * 256:(h + 1) * 256].rearrange(
                        "(f p) n -> p f n", p=128))
            for q in range(4):
                nc.sync.dma_start(
                    out=et8[:, :, q * 256:(q + 1) * 256],
                    in_=et8d[:, q * 256:(q + 1) * 256].rearrange(
                        "(f p) n -> p f n", p=128))
            for h in range(2):
                nc.gpsimd.dma_start(
                    out=vn8[:, h * 4:(h + 1) * 4, :],
                    in_=vn8d[:, h * 4 * F:(h + 1) * 4 * F].rearrange(
                        "p (c f) -> p c f", f=F))

            # remaining streams (slabs of 8 chunks); slabs >=2 are gated on
            # early score matmuls so startup bandwidth goes to slab 0/1.
            gated = []   # (dma_inst, release_chunk_index_in_rb0_ids)
            for sl in range(1, 8):
                n0 = sl * 1024
                d1 = nc.sync.dma_start(
                    out=et8[:, :, n0:n0 + 1024],
                    in_=et8d[:, n0:n0 + 1024].rearrange(
                        "(f p) n -> p f n", p=128))
                d2 = nc.gpsimd.dma_start(
                    out=vn8[:, sl * 8:(sl + 1) * 8, :],
                    in_=vn8d[:, sl * 8 * F:(sl + 1) * 8 * F].rearrange(
                        "p (c f) -> p c f", f=F))
                if sl >= 2:
                    gated.append((d1, (sl - 2) * 8))
                    gated.append((d2, (sl - 2) * 8))
            d = nc.sync.dma_start(
                out=elg8[:, :, 512:NL],
                in_=elg8d[:, 512:NL].rearrange("(f p) n -> p f n", p=128))
            gated.append((d, 16))

            # ---- bf16 diag-band operands (tail of each rb) ----
            etl = persist.tile([128, FC, NL], bf16)
            elg16 = persist.tile([128, FC, NL], bf16)
            vnl = persist.tile([128, OWN, F], bf16)
            d = nc.sync.dma_start(
                out=etl, in_=etld[:, :].rearrange("(f p) n -> p f n", p=128))
            gated.append((d, 24))
            d = nc.sync.dma_start(
                out=elg16, in_=elg16d[:, :].rearrange("(f p) n -> p f n", p=128))
            gated.append((d, 24))
            d = nc.gpsimd.dma_start(
                out=vnl, in_=vnld.rearrange("p (c f) -> p c f", f=F))
            gated.append((d, 24))

            ident = persist.tile([128, 128], bf16)
            nc.scalar.dma_start(out=ident, in_=identd[:, :])
            bv_bc = persist.tile([128, D], f32)
            bv_ap = bvd[:]
            nc.gpsimd.dma_start(out=bv_bc, in_=bass.AP(
                tensor=bv_ap.tensor, offset=bv_ap.offset,
                ap=[[0, 128], *bv_ap.ap]))

            ones_f = persist.tile([128, 1], f32)
            nc.vector.memset(ones_f, 1.0)

            # ---- attention: 2 row-blocks of 512 local rows ----
            for rb in range(RB):
                r0 = rb * 512
                b0 = rb * BCH                   # bf16 band: chunks b0..b0+3
                ids = [c for c in range(CC) if not b0 <= c < b0 + BCH]
                pvt_ps = [
                    ps.tile([128, 512], f32, tag="pvt_ps", bufs=4,
                            name=f"pvt{rb}_{fb}")
                    for fb in range(FC)
                ]
                lacc = work.tile([128, 512], f32, tag="lacc", bufs=1,
                                 name=f"lacc{rb}")
                pt = None
                pend = None      # (pair_base_chunk, pt_tile) awaiting PV
                first_pv = [True]

                def flush_pv(nc=nc, pvt_ps=pvt_ps):
                    cp, tile_ = pend
                    for fb in range(FC):
                        nc.tensor.matmul(
                            pvt_ps[fb],
                            vn8[:, cp:cp + 2, fb * 128:(fb + 1) * 128],
                            tile_,
                            start=first_pv[0], stop=False,
                            perf_mode=DR,
                        )
                    first_pv[0] = False

                for t, c in enumerate(ids):
                    st_ps = ps.tile([128, 512], f32, tag="mm_ps", bufs=3)
                    for kp in range(2):
                        mm = nc.tensor.matmul(
                            st_ps,
                            et8[:, 2 * kp:2 * kp + 2, c * 128:(c + 1) * 128],
                            elg8[:, 2 * kp:2 * kp + 2, r0:r0 + 512],
                            start=(kp == 0), stop=(kp == 1),
                            perf_mode=DR,
                        )
                        if rb == 0 and kp == 0:
                            for dd, rel in gated:
                                if rel == t:
                                    add_dep_helper(
                                        dd.ins, mm.ins,
                                        reason="throttle stream behind compute")
                    if t % 2 == 0:
                        pt = work.tile([128, 2, 512], e5m2, tag="p8", bufs=4)
                    nc.scalar.activation(
                        out=pt[:, t % 2:t % 2 + 1, :], in_=st_ps,
                        func=mybir.ActivationFunctionType.Exp,
                        scale=SCALE, bias=alpha_t[:, c:c + 1],
                    )
                    if t == 0:
                        nc.vector.tensor_copy(out=lacc, in_=pt[:, 0:1, :])
                    else:
                        nc.vector.tensor_add(
                            lacc, lacc, pt[:, t % 2:t % 2 + 1, :])
                    if t % 2 == 1:
                        if pend is not None:
                            flush_pv()
                        pend = (c - 1, pt)
                flush_pv()

                # -- 4 bf16 chunks: this row-block's diagonal band --
                pendb = None
                for k in range(b0, b0 + BCH):
                    st_ps = ps.tile([128, 512], f32, tag="mm_ps", bufs=3)
                    for fc in range(FC):
                        nc.tensor.matmul(
                            st_ps,
                            etl[:, fc:fc + 1, k * 128:(k + 1) * 128],
                            elg16[:, fc:fc + 1, r0:r0 + 512],
                            start=(fc == 0), stop=(fc == FC - 1),
                        )
                    ptb = work.tile([128, 512], bf16, tag="p16", bufs=3)
                    nc.scalar.activation(
                        out=ptb, in_=st_ps,
                        func=mybir.ActivationFunctionType.Exp,
                        scale=SCALE, bias=alpha_t[:, k:k + 1],
                    )
                    nc.vector.tensor_add(lacc, lacc, ptb)
                    if pendb is not None:
                        kk, tile_ = pendb
                        for fb in range(FC):
                            nc.tensor.matmul(
                                pvt_ps[fb],
                                vnl[:, kk:kk + 1, fb * 128:(fb + 1) * 128],
                                tile_,
                                start=False, stop=False,
                            )
                    pendb = (k, ptb)
                kk, tile_ = pendb
                for fb in range(FC):
                    nc.tensor.matmul(
                        pvt_ps[fb],
                        vnl[:, kk:kk + 1, fb * 128:(fb + 1) * 128],
                        tile_,
                        start=False, stop=True,
                    )

                # -- epilogue: L, 1/L, transpose (P V)^T, scale+bias, store --
                l_ps = ps.tile([128, 4], f32, tag="l2", bufs=1,
                               name=f"l_{rb}")
                for jj in range(4):
                    nc.tensor.matmul(
                        l_ps[:, jj:jj + 1],
                        lacc[:, jj * 128:(jj + 1) * 128],
                        ones_f,
                        start=True, stop=True, skip_group_check=True,
                    )
                linv = work.tile([128, 4], f32, tag="linv")
                nc.vector.reciprocal(out=linv, in_=l_ps)

                ptbs = [
                    work.tile([128, 512], bf16, tag="ptb", bufs=8,
                              name=f"ptb{rb}_{fb}")
                    for fb in range(FC)
                ]
                for fb in range(FC):
                    nc.scalar.activation(
                        out=ptbs[fb], in_=pvt_ps[fb],
                        func=mybir.ActivationFunctionType.Copy)
                for jj in range(4):
                    tp_ps = ps.tile([128, 512], bf16, tag="mm_ps", bufs=3,
                                    name=f"tp{rb}_{jj}")
                    for fb in range(FC):
                        nc.tensor.matmul(
                            tp_ps[:, fb * 128:(fb + 1) * 128],
                            ptbs[fb][:, jj * 128:(jj + 1) * 128],
                            ident,
                            start=True, stop=True,
                            is_transpose=True, skip_group_check=True,
                        )
                    o_t = work.tile([128, D], f32, tag="o_t", bufs=3)
                    nc.vector.scalar_tensor_tensor(
                        out=o_t, in0=tp_ps, scalar=linv[:, jj:jj + 1],
                        in1=bv_bc, op0=mybir.AluOpType.mult,
                        op1=mybir.AluOpType.add,
                    )
                    nc.sync.dma_start(
                        out=out[r0 + jj * 128: r0 + (jj + 1) * 128, :],
                        in_=o_t)

    nc.compile()
    return nc


def _get_nc():
    global _NC
    if _NC is None:
        _NC = build_kernel()
    return _NC


def kernel(embedding, W_qk, b_qk, W_v, b_v):
    global LAST_RESULT
    E = np.ascontiguousarray(np.asarray(embedding, dtype=np.float32))
    Wqk = np.asarray(W_qk, dtype=np.float32)
    bqk = np.asarray(b_qk, dtype=np.float32)
    Wv = np.asarray(W_v, dtype=np.float32)
    bv = np.ascontiguousarray(np.asarray(b_v, dtype=np.float32))

    G = Wqk.T @ Wqk
    Et = E @ G                                 # Etilde, f32
    alpha = SCALE * (E @ (Wqk.T @ bqk))        # [N] f32
    V = E @ Wv.T                               # f32 (b_v added on device)

    E8 = E.astype(ml_dtypes.float8_e4m3fn)
    V8 = V.astype(ml_dtypes.float8_e4m3fn)
    ident = np.eye(128, dtype=ml_dtypes.bfloat16)

    in_maps = []
    for i in range(CORES):
        rows = np.arange(i * NL, (i + 1) * NL)
        perm = np.concatenate([rows, np.arange((i + 1) * NL, N),
                               np.arange(0, i * NL)])
        in_maps.append({
            "et8": np.ascontiguousarray(E8[perm].T),
            "vn8": np.ascontiguousarray(
                V8[perm].reshape(CC, 128, F).transpose(1, 0, 2)
            ).reshape(128, CC * F),
            "elg8": np.ascontiguousarray(Et[rows].T).astype(
                ml_dtypes.float8_e4m3fn),
            "etl": np.ascontiguousarray(E[rows].T).astype(ml_dtypes.bfloat16),
            "vnl": np.ascontiguousarray(
                V[rows].astype(ml_dtypes.bfloat16).reshape(
                    OWN, 128, F).transpose(1, 0, 2)
            ).reshape(128, OWN * F),
            "elg16": np.ascontiguousarray(Et[rows].T).astype(
                ml_dtypes.bfloat16),
            "alpha_t": np.ascontiguousarray(
                alpha[perm].reshape(CC, 128).T),
            "ident": ident,
            "bv": bv,
        })

    nc = _get_nc()
    res = run_bass_kernel_spmd(nc, in_maps, core_ids=list(range(CORES)))
    LAST_RESULT = res
    return np.concatenate(
        [np.asarray(res.results[i]["out"]) for i in range(CORES)], axis=0
    )


# revision 25
# speedup vs baseline: 1.6886x; 1.0298x over previous
"""Trainium2 8-core attention kernel v8 (N=8192, D=512, Q==K shared projection).

fp8 DoubleRow formulation on top of the projection-free algebra:

    scores = SCALE * (Etilde E^T) + alpha,   Etilde = E G,  G = W_qk^T W_qk,
    alpha  = SCALE * E (W_qk^T b_qk)         (host-precomputed, f32 exact)
    attn @ V,  V = E W_v^T                   (host-precomputed; b_v added
                                              after row-normalization)

Per core: 1024 output rows; the 64 column chunks are host-ROTATED so the
core's own slab is chunks 0..7.  Q==K makes the score diagonal dominate
the softmax (self-scores 7..14.4 vs off-diag |s|<4.6), so per row-block
only the 4 chunks holding that block's diagonal run in bf16; the other 60
use fp8: e4m3 operands with perf_mode=DoubleRow (2 k-subtiles per
instruction ~2x PE rate), exp output in e5m2 (fp8 P <= ~66, ceiling 57344;
the bf16 diag path holds the exp(14.4)=1.8e6 peaks).

PV(pair k) is emitted after the scores of pair k+1 so the ScalarE exp it
consumes is off the PE critical path.  Row sums accumulate on the idle
DVE; (P V)^T comes back to row-major through 16 tiny PE transposes.
Startup-critical DMA descriptors are split across five engine queues so
the first score tile lands ~12us in.
"""

import ml_dtypes
import numpy as np

import concourse.bass as bass
import concourse.mybir as mybir
import concourse.tile as tile
from concourse import bacc
from concourse.bass_utils import run_bass_kernel_spmd
from concourse.tile_rust import add_dep_helper

N = 8192          # sequence length
F = 512           # features == head dim
D = 512
CORES = 8
NL = N // CORES   # local rows per core (1024)
SCALE = 1.0 / float(np.sqrt(D))

FC = F // 128     # 4 f-chunks
CC = N // 128     # 64 column chunks
OWN = NL // 128   # 8 own-slab chunks
RB = NL // 512    # 2 row-blocks of 512
BCH = 4           # bf16 chunks per row-block (the diag band)

f32 = mybir.dt.float32
bf16 = mybir.dt.bfloat16
e4m3 = mybir.dt.float8e4
e5m2 = mybir.dt.float8e5
DR = mybir.MatmulPerfMode.DoubleRow

_NC = None
LAST_RESULT = None


def build_kernel():
    nc = bacc.Bacc(target_bir_lowering=False)

    et8d = nc.declare_dram_parameter("et8", [F, N], e4m3, isOutput=False)
    vn8d = nc.declare_dram_parameter("vn8", [128, CC * F], e4m3, isOutput=False)
    elg8d = nc.declare_dram_parameter("elg8", [F, NL], e4m3, isOutput=False)
    etld = nc.declare_dram_parameter("etl", [F, NL], bf16, isOutput=False)
    vnld = nc.declare_dram_parameter("vnl", [128, OWN * F], bf16, isOutput=False)
    elg16d = nc.declare_dram_parameter("elg16", [F, NL], bf16, isOutput=False)
    alphad = nc.declare_dram_parameter("alpha_t", [128, CC], f32, isOutput=False)
    identd = nc.declare_dram_parameter("ident", [128, 128], bf16, isOutput=False)
    bvd = nc.declare_dram_parameter("bv", [D], f32, isOutput=False)
    out = nc.declare_dram_parameter("out", [NL, D], f32, isOutput=True)

    with tile.TileContext(nc) as tc:
        with (
            tc.tile_pool(name="persist", bufs=1) as persist,
            tc.tile_pool(name="work", bufs=2) as work,
            tc.tile_pool(name="ps", bufs=2, space="PSUM") as ps,
        ):
            # ---- startup-critical DMAs, spread over engine queues ----
            alpha_t = persist.tile([128, CC], f32)
            et8 = persist.tile([128, FC, N], e4m3)
            vn8 = persist.tile([128, CC, F], e4m3)
            elg8 = persist.tile([128, FC, NL], e4m3)

            nc.scalar.dma_start(out=alpha_t, in_=alphad[:, :])
            for h in range(2):
                nc.scalar.dma_start(
                    out=elg8[:, :, h * 256:(h + 1) * 256],
                    in_=elg8d[:, h * 256:(h + 1) * 256].rearrange(
                        "(f p) n -> p f n", p=128))
            crit0 = None
            for q in range(4):
                d = nc.sync.dma_start(
                    out=et8[:, :, q * 256:(q + 1) * 256],
                    in_=et8d[:, q * 256:(q + 1) * 256].rearrange(
                        "(f p) n -> p f n", p=128))
                if q == 0:
                    crit0 = d
            for h in range(2):
                nc.gpsimd.dma_start(
                    out=vn8[:, h * 4:(h + 1) * 4, :],
                    in_=vn8d[:, h * 4 * F:(h + 1) * 4 * F].rearrange(
                        "p (c f) -> p c f", f=F))

            # remaining streams (slabs of 8 chunks); slabs >=2 are gated on
            # early score matmuls so startup bandwidth goes to slab 0/1.
            gated = []   # (dma_inst, release_chunk_index_in_rb0_ids)
            for sl in range(1, 8):
                n0 = sl * 1024
                d1 = nc.sync.dma_start(
                    out=et8[:, :, n0:n0 + 1024],
                    in_=et8d[:, n0:n0 + 1024].rearrange(
                        "(f p) n -> p f n", p=128))
                d2 = nc.gpsimd.dma_start(
                    out=vn8[:, sl * 8:(sl + 1) * 8, :],
                    in_=vn8d[:, sl * 8 * F:(sl + 1) * 8 * F].rearrange(
                        "p (c f) -> p c f", f=F))
                if sl >= 2:
                    gated.append((d1, 3 * (sl - 2)))
                    gated.append((d2, 3 * (sl - 2)))
            d = nc.sync.dma_start(
                out=elg8[:, :, 512:NL],
                in_=elg8d[:, 512:NL].rearrange("(f p) n -> p f n", p=128))
            gated.append((d, 20))

            # ---- bf16 diag-band operands (tail of each rb) ----
            etl = persist.tile([128, FC, NL], bf16)
            elg16 = persist.tile([128, FC, NL], bf16)
            vnl = persist.tile([128, OWN, F], bf16)
            d = nc.sync.dma_start(
                out=etl, in_=etld[:, :].rearrange("(f p) n -> p f n", p=128))
            gated.append((d, 20))
            d = nc.sync.dma_start(
                out=elg16, in_=elg16d[:, :].rearrange("(f p) n -> p f n", p=128))
            gated.append((d, 20))
            d = nc.gpsimd.dma_start(
                out=vnl, in_=vnld.rearrange("p (c f) -> p c f", f=F))
            gated.append((d, 20))

            ident = persist.tile([128, 128], bf16)
            nc.scalar.dma_start(out=ident, in_=identd[:, :])
            bv_bc = persist.tile([128, D], f32)
            bv_ap = bvd[:]
            nc.gpsimd.dma_start(out=bv_bc, in_=bass.AP(
                tensor=bv_ap.tensor, offset=bv_ap.offset,
                ap=[[0, 128], *bv_ap.ap]))

            ones_f = persist.tile([128, 1], f32)
            nc.vector.memset(ones_f, 1.0)

            # PE warmup: ~3.5us of matmuls on a memset tile, released when
            # the first critical DMA lands, so HAM is at full clock when the
            # real score stream begins.
            wtile = persist.tile([128, 512], bf16)
            nc.vector.memset(wtile, 0.0)
            for w in range(16):
                wm_ps = ps.tile([128, 512], f32, tag="mm_ps", bufs=3)
                wm = nc.tensor.matmul(
                    wm_ps, wtile[:, 0:128], wtile,
                    start=True, stop=True,
                )
                if w == 0:
                    add_dep_helper(wm.ins, crit0.ins,
                                   reason="warmup after first critical dma")

            # ---- attention: 2 row-blocks of 512 local rows ----
            for rb in range(RB):
                r0 = rb * 512
                b0 = rb * BCH                   # bf16 band: chunks b0..b0+3
                ids = [c for c in range(CC) if not b0 <= c < b0 + BCH]
                pvt_ps = [
                    ps.tile([128, 512], f32, tag="pvt_ps", bufs=4,
                            name=f"pvt{rb}_{fb}")
                    for fb in range(FC)
                ]
                lacc = work.tile([128, 512], f32, tag="lacc", bufs=1,
                                 name=f"lacc{rb}")
                pt = None
                pend = None      # (pair_base_chunk, pt_tile) awaiting PV
                first_pv = [True]

                def flush_pv(nc=nc, pvt_ps=pvt_ps):
                    cp, tile_ = pend
                    for fb in range(FC):
                        nc.tensor.matmul(
                            pvt_ps[fb],
                            vn8[:, cp:cp + 2, fb * 128:(fb + 1) * 128],
                            tile_,
                            start=first_pv[0], stop=False,
                            perf_mode=DR,
                        )
                    first_pv[0] = False

                for t, c in enumerate(ids):
                    st_ps = ps.tile([128, 512], f32, tag="mm_ps", bufs=3)
                    for kp in range(2):
                        mm = nc.tensor.matmul(
                            st_ps,
                            et8[:, 2 * kp:2 * kp + 2, c * 128:(c + 1) * 128],
                            elg8[:, 2 * kp:2 * kp + 2, r0:r0 + 512],
                            start=(kp == 0), stop=(kp == 1),
                            perf_mode=DR,
                        )
                        if rb == 0 and kp == 0:
                            for dd, rel in gated:
                                if rel == t:
                                    add_dep_helper(
                                        dd.ins, mm.ins,
                                        reason="throttle stream behind compute")
                    if t % 2 == 0:
                        pt = work.tile([128, 2, 512], e5m2, tag="p8", bufs=4)
                    nc.scalar.activation(
                        out=pt[:, t % 2:t % 2 + 1, :], in_=st_ps,
                        func=mybir.ActivationFunctionType.Exp,
                        scale=SCALE, bias=alpha_t[:, c:c + 1],
                    )
                    if t == 0:
                        nc.vector.tensor_copy(out=lacc, in_=pt[:, 0:1, :])
                    else:
                        nc.vector.tensor_add(
                            lacc, lacc, pt[:, t % 2:t % 2 + 1, :])
                    if t % 2 == 1:
                        if pend is not None:
                            flush_pv()
                        pend = (c - 1, pt)
                flush_pv()

                # -- 4 bf16 chunks: this row-block's diagonal band --
                pendb = None
                for k in range(b0, b0 + BCH):
                    st_ps = ps.tile([128, 512], f32, tag="mm_ps", bufs=3)
                    for fc in range(FC):
                        nc.tensor.matmul(
                            st_ps,
                            etl[:, fc:fc + 1, k * 128:(k + 1) * 128],
                            elg16[:, fc:fc + 1, r0:r0 + 512],
                            start=(fc == 0), stop=(fc == FC - 1),
                        )
                    ptb = work.tile([128, 512], bf16, tag="p16", bufs=3)
                    nc.scalar.activation(
                        out=ptb, in_=st_ps,
                        func=mybir.ActivationFunctionType.Exp,
                        scale=SCALE, bias=alpha_t[:, k:k + 1],
                    )
                    nc.vector.tensor_add(lacc, lacc, ptb)
                    if pendb is not None:
                        kk, tile_ = pendb
                        for fb in range(FC):
                            nc.tensor.matmul(
                                pvt_ps[fb],
                                vnl[:, kk:kk + 1, fb * 128:(fb + 1) * 128],
                                tile_,
                                start=False, stop=False,
                            )
                    pendb = (k, ptb)
                kk, tile_ = pendb
                for fb in range(FC):
                    nc.tensor.matmul(
                        pvt_ps[fb],
                        vnl[:, kk:kk + 1, fb * 128:(fb + 1) * 128],
                        tile_,
                        start=False, stop=True,
                    )

                # -- epilogue: L, 1/L, transpose (P V)^T, scale+bias, store --
                l_ps = ps.tile([128, 4], f32, tag="l2", bufs=1,
                               name=f"l_{rb}")
                for jj in range(4):
                    nc.tensor.matmul(
                        l_ps[:, jj:jj + 1],
                        lacc[:, jj * 128:(jj + 1) * 128],
                        ones_f,
                        start=True, stop=True, skip_group_check=True,
                    )
                linv = work.tile([128, 4], f32, tag="linv")
                nc.vector.reciprocal(out=linv, in_=l_ps)

                ptbs = [
                    work.tile([128, 512], bf16, tag="ptb", bufs=8,
                              name=f"ptb{rb}_{fb}")
                    for fb in range(FC)
                ]
                for fb in range(FC):
                    if fb % 2 == 0:
                        nc.scalar.activation(
                            out=ptbs[fb], in_=pvt_ps[fb],
                            func=mybir.ActivationFunctionType.Copy)
                    else:
                        nc.vector.tensor_copy(
                            out=ptbs[fb], in_=pvt_ps[fb])
                for jj in range(4):
                    tp_ps = ps.tile([128, 512], bf16, tag="mm_ps", bufs=3,
                                    name=f"tp{rb}_{jj}")
                    for fb in range(FC):
                        nc.tensor.matmul(
                            tp_ps[:, fb * 128:(fb + 1) * 128],
                            ptbs[fb][:, jj * 128:(jj + 1) * 128],
                            ident,
                            start=True, stop=True,
                            is_transpose=True, skip_group_check=True,
                        )
                    o_t = work.tile([128, D], f32, tag="o_t", bufs=3)
                    nc.vector.scalar_tensor_tensor(
                        out=o_t, in0=tp_ps, scalar=linv[:, jj:jj + 1],
                        in1=bv_bc, op0=mybir.AluOpType.mult,
                        op1=mybir.AluOpType.add,
                    )
                    nc.sync.dma_start(
                        out=out[r0 + jj * 128: r0 + (jj + 1) * 128, :],
                        in_=o_t)

    nc.compile()
    return nc


def _get_nc():
    global _NC
    if _NC is None:
        _NC = build_kernel()
    return _NC


def kernel(embedding, W_qk, b_qk, W_v, b_v):
    global LAST_RESULT
    E = np.ascontiguousarray(np.asarray(embedding, dtype=np.float32))
    Wqk = np.asarray(W_qk, dtype=np.float32)
    bqk = np.asarray(b_qk, dtype=np.float32)
    Wv = np.asarray(W_v, dtype=np.float32)
    bv = np.ascontiguousarray(np.asarray(b_v, dtype=np.float32))

    G = Wqk.T @ Wqk
    Et = E @ G                                 # Etilde, f32
    alpha = SCALE * (E @ (Wqk.T @ bqk))        # [N] f32
    V = E @ Wv.T                               # f32 (b_v added on device)

    E8 = E.astype(ml_dtypes.float8_e4m3fn)
    V8 = V.astype(ml_dtypes.float8_e4m3fn)
    ident = np.eye(128, dtype=ml_dtypes.bfloat16)

    in_maps = []
    for i in range(CORES):
        rows = np.arange(i * NL, (i + 1) * NL)
        perm = np.concatenate([rows, np.arange((i + 1) * NL, N),
                               np.arange(0, i * NL)])
        in_maps.append({
            "et8": np.ascontiguousarray(E8[perm].T),
            "vn8": np.ascontiguousarray(
                V8[perm].reshape(CC, 128, F).transpose(1, 0, 2)
            ).reshape(128, CC * F),
            "elg8": np.ascontiguousarray(Et[rows].T).astype(
                ml_dtypes.float8_e4m3fn),
            "etl": np.ascontiguousarray(E[rows].T).astype(ml_dtypes.bfloat16),
            "vnl": np.ascontiguousarray(
                V[rows].astype(ml_dtypes.bfloat16).reshape(
                    OWN, 128, F).transpose(1, 0, 2)
            ).reshape(128, OWN * F),
            "elg16": np.ascontiguousarray(Et[rows].T).astype(
                ml_dtypes.bfloat16),
            "alpha_t": np.ascontiguousarray(
                alpha[perm].reshape(CC, 128).T),
            "ident": ident,
            "bv": bv,
        })

    nc = _get_nc()
    res = run_bass_kernel_spmd(nc, in_maps, core_ids=list(range(CORES)))
    LAST_RESULT = res
    return np.concatenate(
        [np.asarray(res.results[i]["out"]) for i in range(CORES)], axis=0
    )
